# revision 3
# baseline (speedup 1.0000x reference)
"""Multi-head attention block (QKV proj + softmax attention + out proj) on 8
Trainium2 NeuronCores, data-parallel over the batch dimension (one batch
element per core).

Self-contained: hardcodes shapes for x [8, 1024, 768], qkv_w [768, 2304],
proj_w [768, 768], proj_b [768]; returns [8, 1024, 768] float32.

v2 design notes (why this is structured the way it is):
- bf16 matmul operands everywhere: weight loads use the background weight
  buffer / FWL so LDWEIGHTS hides under the matmul stream (f32r serializes
  ~180ns/MM), and input DMA bytes halve.
- The exp() of the 12.6M attention scores runs only on ScalarE
  ((N+352)/1.2ns per activate => ~110us total). That is co-bottleneck with
  the PE (~123us), so the QKV matmuls for head-pair hp+1 are emitted
  interleaved with the scores/AV of hp: ScalarE starts exping ~30us into
  the kernel and never starves.
- Softmax denominator comes for free as a 65th V_AUG row; the normalize is
  DVE reciprocal + GpSimd partition_broadcast + GpSimd multiply (the old
  DRAM-roundtrip broadcast exposed ~15us after the last head-pair).
- Scores matmuls use K=64 row tiles (head A rows 0-63, head B 64-127)
  emitted adjacently so both heads stream through the PE concurrently.
"""

import numpy as np
import ml_dtypes

import concourse.bass as bass
import concourse.mybir as mybir
import concourse.tile as tile
from concourse import bacc

N_CORES = 8
N = 1024          # tokens per batch element
C = 768           # model dim
H = 12            # heads
HD = 64           # head dim
CT = C // 128     # 6 contraction tiles
TT = N // 128     # 8 token tiles
SCALE = HD ** -0.5

F32 = mybir.dt.float32
BF16 = mybir.dt.bfloat16


def _build():
    nc = bacc.Bacc("TRN2", target_bir_lowering=False, debug=False,
                   num_devices=N_CORES)
    x_t = nc.dram_tensor("x_t", [C, N], BF16, kind="ExternalInput").ap()
    # wqk columns host-reordered: per head-pair hp, cols [hp*256, hp*256+128)
    # are Q features, [hp*256+128, (hp+1)*256) are K features.
    wqk = nc.dram_tensor("wqk", [C, 2 * C], BF16, kind="ExternalInput").ap()
    wv = nc.dram_tensor("wv", [C, C], BF16, kind="ExternalInput").ap()
    pw = nc.dram_tensor("pw", [C, C], BF16, kind="ExternalInput").ap()
    pb = nc.dram_tensor("pb", [1, C], F32, kind="ExternalInput").ap()
    out = nc.dram_tensor("out", [N, C], F32, kind="ExternalOutput").ap()

    with tile.TileContext(nc) as tc:
        _emit(nc, tc, x_t, wqk, wv, pw, pb, out)
    nc.compile()
    return nc


def _emit(nc, tc, x_t, wqk, wv, pw, pb, out):
    from contextlib import ExitStack
    ctx = ExitStack()
    with ctx:
        xt_pool = ctx.enter_context(tc.tile_pool(name="xt", bufs=1))
        wqk_pool = ctx.enter_context(tc.tile_pool(name="wqk", bufs=1))
        wv_pool = ctx.enter_context(tc.tile_pool(name="wv", bufs=1))
        pw_pool = ctx.enter_context(tc.tile_pool(name="pw", bufs=1))
        qk_pool = ctx.enter_context(tc.tile_pool(name="qk", bufs=1))
        vaug_pool = ctx.enter_context(tc.tile_pool(name="vaug", bufs=1))
        outt_pool = ctx.enter_context(tc.tile_pool(name="outt", bufs=1))
        exps_pool = ctx.enter_context(tc.tile_pool(name="exps", bufs=6))
        nrm_pool = ctx.enter_context(tc.tile_pool(name="nrm", bufs=2))
        rcp_pool = ctx.enter_context(tc.tile_pool(name="rcp", bufs=2))
        bc_pool = ctx.enter_context(tc.tile_pool(name="bc", bufs=2))
        const_pool = ctx.enter_context(tc.tile_pool(name="const", bufs=1))
        outsb_pool = ctx.enter_context(tc.tile_pool(name="outsb", bufs=2))

        # ---- input DMAs, spread over idle engine queues, large packets ----
        # Wv first (gates the V matmuls), per-ct blocks: 1.5KB packets.
        WV = wv_pool.tile([128, CT, C], BF16, tag="wv")
        for ct in range(CT):
            nc.sync.dma_start(WV[:, ct, :], wv[ct * 128:(ct + 1) * 128, :])
        # XT in two token halves on the scalar queue (runs parallel to Wv;
        # scalar's exp work only starts much later).
        XT = xt_pool.tile([128, CT, N], BF16, tag="xt")
        for h in range(2):
            nc.scalar.dma_start(
                XT[:, :, h * 512:(h + 1) * 512],
                x_t[:, h * 512:(h + 1) * 512].rearrange("(c p) n -> p c n", p=128))
        # WQK per head-pair-pair blocks (512 cols = 1KB packets), gpsimd queue.
        WQK = wqk_pool.tile([128, CT, 2 * C], BF16, tag="wqk")
        for g in range(3):
            nc.gpsimd.dma_start(
                WQK[:, :, g * 512:(g + 1) * 512],
                wqk[:, g * 512:(g + 1) * 512].rearrange("(c p) f -> p c f", p=128))
        # proj weights + bias (needed only ~100us in).
        PW = pw_pool.tile([128, CT, C], BF16, tag="pw")
        nc.scalar.dma_start(PW[:], pw.rearrange("(c p) f -> p c f", p=128))
        pbb = const_pool.tile([128, C], F32, tag="pb")
        pb_src = pb[:, :]
        pb_bcast = bass.AP(tensor=pb_src.tensor, offset=pb_src.offset,
                           ap=[[0, 128]] + [list(a) for a in pb_src.ap[1:]])
        nc.gpsimd.dma_start(pbb[:], pb_bcast)

        ones_bf = const_pool.tile([128, 96], BF16, tag="ones")
        nc.vector.memset(ones_bf[:], 1.0)
        V_AUG = vaug_pool.tile([128, TT, H, HD + 1], BF16, tag="vaug")
        nc.vector.tensor_copy(
            V_AUG[:, :, :, HD:HD + 1].rearrange("p t h one -> p (t h one)"),
            ones_bf[:])

        QT = qk_pool.tile([128, CT, N], BF16, tag="qt")
        KT = qk_pool.tile([128, CT, N], BF16, tag="kt")
        outT = outt_pool.tile([128, CT, N], BF16, tag="outt")

        sc_ps = ctx.enter_context(tc.tile_pool(name="scps", bufs=2, space="PSUM"))
        av_ps = ctx.enter_context(tc.tile_pool(name="avps", bufs=2, space="PSUM"))

        # ---- emit helpers ----
        def emit_v(tt):
            ps = sc_ps.tile([128, 1024], F32, tag="sc")
            for w0, wn in ((0, 512), (512, 256)):
                for ct in range(CT):
                    nc.tensor.matmul(
                        ps[:, w0:w0 + wn],
                        lhsT=XT[:, ct, tt * 128:(tt + 1) * 128],
                        rhs=WV[:, ct, w0:w0 + wn],
                        start=(ct == 0), stop=(ct == CT - 1))
            nc.vector.tensor_copy(
                V_AUG[:, tt, :, 0:HD],
                ps[:, :C].rearrange("p (h d) -> p h d", d=HD))

        def emit_qk(hp, which):
            # which: 0 => Q chunk of head-pair hp, 1 => K chunk
            dest = QT if which == 0 else KT
            f0 = hp * 256 + which * 128
            ps = sc_ps.tile([128, 1024], F32, tag="sc")
            for ct in range(CT):
                for qc in range(2):
                    nc.tensor.matmul(
                        ps[:, qc * 512:(qc + 1) * 512],
                        lhsT=WQK[:, ct, f0:f0 + 128],
                        rhs=XT[:, ct, qc * 512:(qc + 1) * 512],
                        start=(ct == 0), stop=(ct == CT - 1))
            nc.vector.tensor_copy(dest[:, hp, :], ps[:])

        def emit_scores(hp, kt):
            psA = sc_ps.tile([128, 1024], F32, tag="sc")
            psB = sc_ps.tile([128, 1024], F32, tag="sc")
            for qc in range(2):
                nc.tensor.matmul(
                    psA[:, qc * 512:(qc + 1) * 512],
                    lhsT=KT[0:64, hp, kt * 128:(kt + 1) * 128],
                    rhs=QT[0:64, hp, qc * 512:(qc + 1) * 512],
                    start=True, stop=True)
                nc.tensor.matmul(
                    psB[:, qc * 512:(qc + 1) * 512],
                    lhsT=KT[64:128, hp, kt * 128:(kt + 1) * 128],
                    rhs=QT[64:128, hp, qc * 512:(qc + 1) * 512],
                    start=True, stop=True)
            eA = exps_pool.tile([128, N], BF16, tag="exps")
            eB = exps_pool.tile([128, N], BF16, tag="exps")
            nc.scalar.activation(eA[:], psA[:], mybir.ActivationFunctionType.Exp,
                                 scale=SCALE)
            nc.scalar.activation(eB[:], psB[:], mybir.ActivationFunctionType.Exp,
                                 scale=SCALE)
            return eA, eB

        def emit_av(hp, kt, eA, eB, avA, avB):
            for av, e, head in ((avA, eA, 2 * hp), (avB, eB, 2 * hp + 1)):
                for qc in range(2):
                    nc.tensor.matmul(
                        av[:, qc * 512:(qc + 1) * 512],
                        lhsT=V_AUG[:, kt, head, :],
                        rhs=e[:, qc * 512:(qc + 1) * 512],
                        start=(kt == 0), stop=(kt == TT - 1))

        def emit_normalize(hp, avA, avB):
            for av, poff in ((avA, 0), (avB, 64)):
                U = nrm_pool.tile([HD + 1, N], F32, tag="U")
                nc.vector.tensor_copy(U[:], av[:])
                r = rcp_pool.tile([1, N], F32, tag="r")
                nc.vector.reciprocal(r[:], U[HD:HD + 1, :])
                bcst = bc_pool.tile([64, N], F32, tag="bc")
                nc.gpsimd.partition_broadcast(bcst[:], r[0:1, :], channels=64)
                nc.gpsimd.tensor_mul(
                    outT[poff:poff + 64, hp, :], U[0:HD, :], bcst[:])

        def emit_proj(tt):
            ps = sc_ps.tile([128, 1024], F32, tag="sc")
            for ct in range(CT):
                for nch in range(2):
                    nc.tensor.matmul(
                        ps[:, nch * 512:nch * 512 + 384],
                        lhsT=outT[:, ct, tt * 128:(tt + 1) * 128],
                        rhs=PW[:, ct, nch * 384:(nch + 1) * 384],
                        start=(ct == 0), stop=(ct == CT - 1))
            osb = outsb_pool.tile([128, C], F32, tag="outsb")
            ps_v = ps[:].rearrange("p (c x) -> p c x", c=2)[:, :, 0:384]
            osb_v = osb[:].rearrange("p (c x) -> p c x", c=2)
            pbb_v = pbb[:].rearrange("p (c x) -> p c x", c=2)
            nc.vector.tensor_add(osb_v, ps_v, pbb_v)
            nc.sync.dma_start(out[tt * 128:(tt + 1) * 128, :], osb[:])

        # ---- emission schedule ----
        for tt in range(TT):
            emit_v(tt)
        emit_qk(0, 0)
        emit_qk(0, 1)
        for hp in range(CT):
            avA = av_ps.tile([HD + 1, 1024], F32, tag="av")
            avB = av_ps.tile([HD + 1, 1024], F32, tag="av")
            pend = []
            for kt in range(TT):
                eA, eB = emit_scores(hp, kt)
                pend.append((hp, kt, eA, eB, avA, avB))
                if kt == 1 and hp + 1 < CT:
                    emit_qk(hp + 1, 0)
                if kt == 4 and hp + 1 < CT:
                    emit_qk(hp + 1, 1)
                if len(pend) >= 3:
                    emit_av(*pend.pop(0))
            for args in pend:
                emit_av(*args)
            emit_normalize(hp, avA, avB)
        for tt in range(TT):
            emit_proj(tt)


_CACHE = {}


def _get_runner():
    """Build + compile once; return a callable(in_maps) -> list of out dicts.

    Keeps a persistent jitted shard_map executable so repeat calls skip
    retracing/recompiling (mirrors bass2jax.run_bass_via_pjrt).
    """
    if "runner" in _CACHE:
        return _CACHE["runner"]

    import jax
    from jax.experimental.shard_map import shard_map
    from jax.sharding import Mesh, PartitionSpec
    from concourse import bass2jax

    nc = _build()
    bass2jax.install_neuronx_cc_hook()

    partition_name = (nc.partition_id_tensor.name if nc.partition_id_tensor
                      else None)
    in_names, out_names, out_avals, zero_outs = [], [], [], []
    for alloc in nc.m.functions[0].allocations:
        if not isinstance(alloc, mybir.MemoryLocationSet):
            continue
        name = alloc.memorylocations[0].name
        if alloc.kind == "ExternalInput":
            if name != partition_name:
                in_names.append(name)
        elif alloc.kind == "ExternalOutput":
            out_names.append(name)
            shape = tuple(alloc.tensor_shape)
            dtype = mybir.dt.np(alloc.dtype)
            out_avals.append(jax.core.ShapedArray(shape, dtype))
            zero_outs.append(np.zeros(shape, dtype))
    n_params = len(in_names)
    n_outs = len(out_avals)
    all_in_names = list(in_names) + list(out_names)
    if partition_name is not None:
        all_in_names.append(partition_name)
    donate = tuple(range(n_params, n_params + n_outs))

    def _body(*args):
        operands = list(args)
        if partition_name is not None:
            operands.append(bass2jax.partition_id_tensor())
        outs = bass2jax._bass_exec_p.bind(
            *operands,
            out_avals=tuple(out_avals),
            in_names=tuple(all_in_names),
            out_names=tuple(out_names),
            lowering_input_output_aliases=(),
            sim_require_finite=True,
            sim_require_nnan=True,
            nc=nc,
        )
        return tuple(outs)

    devices = jax.devices()[:N_CORES]
    mesh = Mesh(np.asarray(devices), ("core",))
    in_specs = (PartitionSpec("core"),) * (n_params + n_outs)
    out_specs = (PartitionSpec("core"),) * n_outs
    sharded = jax.jit(
        shard_map(_body, mesh=mesh, in_specs=in_specs, out_specs=out_specs,
                  check_rep=False),
        donate_argnums=donate, keep_unused=True)

    def runner(in_maps):
        concat_in = [
            np.concatenate([np.asarray(m[name]) for m in in_maps], axis=0)
            for name in in_names
        ]
        concat_zeros = [
            np.zeros((N_CORES * z.shape[0], *z.shape[1:]), z.dtype)
            for z in zero_outs
        ]
        out_arrs = sharded(*concat_in, *concat_zeros)
        return [
            {name: np.asarray(out_arrs[i]).reshape(N_CORES, *out_avals[i].shape)[c]
             for i, name in enumerate(out_names)}
            for c in range(N_CORES)
        ]

    _CACHE["runner"] = runner
    _CACHE["nc"] = nc
    return runner


def make_in_maps(x, qkv_w, proj_w, proj_b):
    bf16 = ml_dtypes.bfloat16
    qkv_w = np.ascontiguousarray(np.asarray(qkv_w, dtype=np.float32))
    # reorder Q|K columns into per-head-pair [Q_hp(128) | K_hp(128)] blocks
    wqk_i = np.empty((C, 2 * C), dtype=np.float32)
    for hp in range(CT):
        wqk_i[:, hp * 256:hp * 256 + 128] = qkv_w[:, hp * 128:(hp + 1) * 128]
        wqk_i[:, hp * 256 + 128:(hp + 1) * 256] = \
            qkv_w[:, C + hp * 128:C + (hp + 1) * 128]
    wqk_i = wqk_i.astype(bf16)
    wv = qkv_w[:, 2 * C:3 * C].astype(bf16)
    pw_b = np.asarray(proj_w, dtype=np.float32).astype(bf16)
    pb = np.asarray(proj_b, dtype=np.float32).reshape(1, C)
    return [
        {
            "x_t": np.ascontiguousarray(
                np.asarray(x[b], dtype=np.float32).T).astype(bf16),
            "wqk": wqk_i,
            "wv": wv,
            "pw": pw_b,
            "pb": pb,
        }
        for b in range(N_CORES)
    ]


def kernel(x, qkv_w, proj_w, proj_b):
    runner = _get_runner()
    results = runner(make_in_maps(x, qkv_w, proj_w, proj_b))
    return np.stack([results[b]["out"] for b in range(N_CORES)], axis=0)


# revision 8
# speedup vs baseline: 1.0781x; 1.0781x over previous
"""Multi-head attention block (QKV proj + softmax attention + out proj) on 8
Trainium2 NeuronCores, data-parallel over the batch dimension (one batch
element per core).

Self-contained: hardcodes shapes for x [8, 1024, 768], qkv_w [768, 2304],
proj_w [768, 768], proj_b [768]; returns [8, 1024, 768] float32.

v2 design notes (why this is structured the way it is):
- bf16 matmul operands everywhere: weight loads use the background weight
  buffer / FWL so LDWEIGHTS hides under the matmul stream (f32r serializes
  ~180ns/MM), and input DMA bytes halve.
- The exp() of the 12.6M attention scores runs only on ScalarE
  ((N+352)/1.2ns per activate => ~110us total). That is co-bottleneck with
  the PE (~123us), so the QKV matmuls for head-pair hp+1 are emitted
  interleaved with the scores/AV of hp: ScalarE starts exping ~30us into
  the kernel and never starves.
- Softmax denominator comes for free as a 65th V_AUG row; the normalize is
  DVE reciprocal + GpSimd partition_broadcast + GpSimd multiply (the old
  DRAM-roundtrip broadcast exposed ~15us after the last head-pair).
- Scores matmuls use K=64 row tiles (head A rows 0-63, head B 64-127)
  emitted adjacently so both heads stream through the PE concurrently.
"""

import numpy as np
import ml_dtypes

import concourse.bass as bass
import concourse.mybir as mybir
import concourse.tile as tile
from concourse import bacc

N_CORES = 8
N = 1024          # tokens per batch element
C = 768           # model dim
H = 12            # heads
HD = 64           # head dim
CT = C // 128     # 6 contraction tiles
TT = N // 128     # 8 token tiles
SCALE = HD ** -0.5

F32 = mybir.dt.float32
BF16 = mybir.dt.bfloat16


def _build():
    nc = bacc.Bacc("TRN2", target_bir_lowering=False, debug=False,
                   num_devices=N_CORES)
    x_t = nc.dram_tensor("x_t", [C, N], BF16, kind="ExternalInput").ap()
    # wqk columns host-reordered: per head-pair hp, cols [hp*256, hp*256+128)
    # are Q features, [hp*256+128, (hp+1)*256) are K features.
    wqk = nc.dram_tensor("wqk", [C, 2 * C], BF16, kind="ExternalInput").ap()
    wv = nc.dram_tensor("wv", [C, C], BF16, kind="ExternalInput").ap()
    pw = nc.dram_tensor("pw", [C, C], BF16, kind="ExternalInput").ap()
    pb = nc.dram_tensor("pb", [1, C], F32, kind="ExternalInput").ap()
    out = nc.dram_tensor("out", [N, C], F32, kind="ExternalOutput").ap()

    with tile.TileContext(nc) as tc:
        _emit(nc, tc, x_t, wqk, wv, pw, pb, out)
    nc.compile()
    return nc


def _emit(nc, tc, x_t, wqk, wv, pw, pb, out):
    from contextlib import ExitStack
    ctx = ExitStack()
    with ctx:
        xt_pool = ctx.enter_context(tc.tile_pool(name="xt", bufs=1))
        wqk_pool = ctx.enter_context(tc.tile_pool(name="wqk", bufs=1))
        wv_pool = ctx.enter_context(tc.tile_pool(name="wv", bufs=1))
        pw_pool = ctx.enter_context(tc.tile_pool(name="pw", bufs=1))
        qk_pool = ctx.enter_context(tc.tile_pool(name="qk", bufs=1))
        vaug_pool = ctx.enter_context(tc.tile_pool(name="vaug", bufs=1))
        outt_pool = ctx.enter_context(tc.tile_pool(name="outt", bufs=1))
        exps_pool = ctx.enter_context(tc.tile_pool(name="exps", bufs=14))
        nrm_pool = ctx.enter_context(tc.tile_pool(name="nrm", bufs=2))
        rcp_pool = ctx.enter_context(tc.tile_pool(name="rcp", bufs=2))
        bc_pool = ctx.enter_context(tc.tile_pool(name="bc", bufs=2))
        const_pool = ctx.enter_context(tc.tile_pool(name="const", bufs=1))
        outsb_pool = ctx.enter_context(tc.tile_pool(name="outsb", bufs=2))

        # ---- input DMAs, spread over idle engine queues, large packets ----
        # Wv first (gates the V matmuls), per-ct blocks: 1.5KB packets.
        WV = wv_pool.tile([128, CT, C], BF16, tag="wv")
        for ct in range(CT):
            nc.sync.dma_start(WV[:, ct, :], wv[ct * 128:(ct + 1) * 128, :])
        # XT in two token halves on the scalar queue (runs parallel to Wv;
        # scalar's exp work only starts much later).
        XT = xt_pool.tile([128, CT, N], BF16, tag="xt")
        for h in range(2):
            nc.scalar.dma_start(
                XT[:, :, h * 512:(h + 1) * 512],
                x_t[:, h * 512:(h + 1) * 512].rearrange("(c p) n -> p c n", p=128))
        # WQK per head-pair-pair blocks (512 cols = 1KB packets), gpsimd queue.
        WQK = wqk_pool.tile([128, CT, 2 * C], BF16, tag="wqk")
        for g in range(3):
            nc.gpsimd.dma_start(
                WQK[:, :, g * 512:(g + 1) * 512],
                wqk[:, g * 512:(g + 1) * 512].rearrange("(c p) f -> p c f", p=128))
        # proj weights + bias (needed only ~100us in).
        PW = pw_pool.tile([128, CT, C], BF16, tag="pw")
        nc.scalar.dma_start(PW[:], pw.rearrange("(c p) f -> p c f", p=128))
        pbb = const_pool.tile([128, C], F32, tag="pb")
        pb_src = pb[:, :]
        pb_bcast = bass.AP(tensor=pb_src.tensor, offset=pb_src.offset,
                           ap=[[0, 128]] + [list(a) for a in pb_src.ap[1:]])
        nc.gpsimd.dma_start(pbb[:], pb_bcast)

        ones_bf = const_pool.tile([128, 96], BF16, tag="ones")
        nc.vector.memset(ones_bf[:], 1.0)
        V_AUG = vaug_pool.tile([128, TT, H, HD + 1], BF16, tag="vaug")
        nc.vector.tensor_copy(
            V_AUG[:, :, :, HD:HD + 1].rearrange("p t h one -> p (t h one)"),
            ones_bf[:])

        QT = qk_pool.tile([128, CT, N], BF16, tag="qt")
        KT = qk_pool.tile([128, CT, N], BF16, tag="kt")
        outT = outt_pool.tile([128, CT, N], BF16, tag="outt")

        sc_ps = ctx.enter_context(tc.tile_pool(name="scps", bufs=2, space="PSUM"))
        av_ps = ctx.enter_context(tc.tile_pool(name="avps", bufs=2, space="PSUM"))

        # ---- emit helpers ----
        def emit_v(tt):
            # V matmuls use the av PSUM pool: keeps the scores pool rotation
            # free of V-eviction dependencies, and the av slots aren't needed
            # until the first AV accumulation (after all V tiles retire).
            ps = av_ps.tile([128, 1024], F32, tag="av")
            for w0, wn in ((0, 512), (512, 256)):
                for ct in range(CT):
                    nc.tensor.matmul(
                        ps[:, w0:w0 + wn],
                        lhsT=XT[:, ct, tt * 128:(tt + 1) * 128],
                        rhs=WV[:, ct, w0:w0 + wn],
                        start=(ct == 0), stop=(ct == CT - 1))
            nc.vector.tensor_copy(
                V_AUG[:, tt, :, 0:HD],
                ps[:, :C].rearrange("p (h d) -> p h d", d=HD))

        def emit_qk(hp, which):
            # which: 0 => Q chunk of head-pair hp, 1 => K chunk
            dest = QT if which == 0 else KT
            f0 = hp * 256 + which * 128
            ps = sc_ps.tile([128, 1024], F32, tag="sc")
            for ct in range(CT):
                for qc in range(2):
                    nc.tensor.matmul(
                        ps[:, qc * 512:(qc + 1) * 512],
                        lhsT=WQK[:, ct, f0:f0 + 128],
                        rhs=XT[:, ct, qc * 512:(qc + 1) * 512],
                        start=(ct == 0), stop=(ct == CT - 1))
            nc.vector.tensor_copy(dest[:, hp, :], ps[:])

        def emit_scores(hp, kt):
            psA = sc_ps.tile([128, 1024], F32, tag="sc")
            psB = sc_ps.tile([128, 1024], F32, tag="sc")
            for qc in range(2):
                nc.tensor.matmul(
                    psA[:, qc * 512:(qc + 1) * 512],
                    lhsT=KT[0:64, hp, kt * 128:(kt + 1) * 128],
                    rhs=QT[0:64, hp, qc * 512:(qc + 1) * 512],
                    start=True, stop=True)
                nc.tensor.matmul(
                    psB[:, qc * 512:(qc + 1) * 512],
                    lhsT=KT[64:128, hp, kt * 128:(kt + 1) * 128],
                    rhs=QT[64:128, hp, qc * 512:(qc + 1) * 512],
                    start=True, stop=True)
            eA = exps_pool.tile([128, N], BF16, tag="exps")
            eB = exps_pool.tile([128, N], BF16, tag="exps")
            nc.scalar.activation(eA[:], psA[:], mybir.ActivationFunctionType.Exp,
                                 scale=SCALE)
            nc.scalar.activation(eB[:], psB[:], mybir.ActivationFunctionType.Exp,
                                 scale=SCALE)
            return eA, eB

        def emit_av(hp, kt, eA, eB, avA, avB):
            for av, e, head in ((avA, eA, 2 * hp), (avB, eB, 2 * hp + 1)):
                for qc in range(2):
                    nc.tensor.matmul(
                        av[:, qc * 512:(qc + 1) * 512],
                        lhsT=V_AUG[:, kt, head, :],
                        rhs=e[:, qc * 512:(qc + 1) * 512],
                        start=(kt == 0), stop=(kt == TT - 1))

        def emit_normalize(hp, avA, avB):
            # DVE order: copyA, recipA, copyB, recipB, mulA, mulB — the gpsimd
            # broadcast for A runs during copyB/recipB so mulA never waits.
            Us, bcs = [], []
            for av in (avA, avB):
                U = nrm_pool.tile([HD + 1, N], F32, tag="U")
                nc.vector.tensor_copy(U[:], av[:])
                r = rcp_pool.tile([1, N], F32, tag="r")
                nc.vector.reciprocal(r[:], U[HD:HD + 1, :])
                bcst = bc_pool.tile([64, N], F32, tag="bc")
                nc.gpsimd.partition_broadcast(bcst[:], r[0:1, :], channels=64)
                Us.append(U)
                bcs.append(bcst)
            for U, bcst, poff in ((Us[0], bcs[0], 0), (Us[1], bcs[1], 64)):
                nc.vector.tensor_mul(
                    outT[poff:poff + 64, hp, :], U[0:HD, :], bcst[:])

        def emit_proj(tt):
            ps = sc_ps.tile([128, 1024], F32, tag="sc")
            for ct in range(CT):
                for nch in range(2):
                    nc.tensor.matmul(
                        ps[:, nch * 512:nch * 512 + 384],
                        lhsT=outT[:, ct, tt * 128:(tt + 1) * 128],
                        rhs=PW[:, ct, nch * 384:(nch + 1) * 384],
                        start=(ct == 0), stop=(ct == CT - 1))
            osb = outsb_pool.tile([128, C], F32, tag="outsb")
            ps_v = ps[:].rearrange("p (c x) -> p c x", c=2)[:, :, 0:384]
            osb_v = osb[:].rearrange("p (c x) -> p c x", c=2)
            pbb_v = pbb[:].rearrange("p (c x) -> p c x", c=2)
            nc.vector.tensor_add(osb_v, ps_v, pbb_v)
            nc.sync.dma_start(out[tt * 128:(tt + 1) * 128, :], osb[:])

        # ---- emission schedule ----
        # QK for head-pair 0 first so ScalarE's exp stream starts ~18us in;
        # the V matmuls interleave into hp0's score iterations (kts 0-3).
        # AV for hp0 is held back (pend>=5) until all V tiles are emitted,
        # else the AV matmul would head-block the PE FIFO on V_AUG writes
        # queued behind it.
        emit_qk(0, 0)
        emit_qk(0, 1)
        for hp in range(CT):
            avA = avB = None
            pend = []
            thresh = 5 if hp == 0 else 3
            for kt in range(TT):
                eA, eB = emit_scores(hp, kt)
                pend.append((hp, kt, eA, eB))
                if hp == 0:
                    if kt < 4:
                        emit_v(2 * kt)
                        emit_v(2 * kt + 1)
                    if kt == 4 and hp + 1 < CT:
                        emit_qk(hp + 1, 0)
                    if kt == 6 and hp + 1 < CT:
                        emit_qk(hp + 1, 1)
                else:
                    if kt == 1 and hp + 1 < CT:
                        emit_qk(hp + 1, 0)
                    if kt == 4 and hp + 1 < CT:
                        emit_qk(hp + 1, 1)
                if len(pend) >= thresh:
                    if avA is None:
                        avA = av_ps.tile([HD + 1, 1024], F32, tag="av")
                        avB = av_ps.tile([HD + 1, 1024], F32, tag="av")
                    emit_av(*pend.pop(0), avA, avB)
            if avA is None:
                avA = av_ps.tile([HD + 1, 1024], F32, tag="av")
                avB = av_ps.tile([HD + 1, 1024], F32, tag="av")
            for args in pend:
                emit_av(*args, avA, avB)
            emit_normalize(hp, avA, avB)
        for tt in range(TT):
            emit_proj(tt)


_CACHE = {}


def _get_runner():
    """Build + compile once; return a callable(in_maps) -> list of out dicts.

    Keeps a persistent jitted shard_map executable so repeat calls skip
    retracing/recompiling (mirrors bass2jax.run_bass_via_pjrt).
    """
    if "runner" in _CACHE:
        return _CACHE["runner"]

    import jax
    from jax.experimental.shard_map import shard_map
    from jax.sharding import Mesh, PartitionSpec
    from concourse import bass2jax

    nc = _build()
    bass2jax.install_neuronx_cc_hook()

    partition_name = (nc.partition_id_tensor.name if nc.partition_id_tensor
                      else None)
    in_names, out_names, out_avals, zero_outs = [], [], [], []
    for alloc in nc.m.functions[0].allocations:
        if not isinstance(alloc, mybir.MemoryLocationSet):
            continue
        name = alloc.memorylocations[0].name
        if alloc.kind == "ExternalInput":
            if name != partition_name:
                in_names.append(name)
        elif alloc.kind == "ExternalOutput":
            out_names.append(name)
            shape = tuple(alloc.tensor_shape)
            dtype = mybir.dt.np(alloc.dtype)
            out_avals.append(jax.core.ShapedArray(shape, dtype))
            zero_outs.append(np.zeros(shape, dtype))
    n_params = len(in_names)
    n_outs = len(out_avals)
    all_in_names = list(in_names) + list(out_names)
    if partition_name is not None:
        all_in_names.append(partition_name)
    donate = tuple(range(n_params, n_params + n_outs))

    def _body(*args):
        operands = list(args)
        if partition_name is not None:
            operands.append(bass2jax.partition_id_tensor())
        outs = bass2jax._bass_exec_p.bind(
            *operands,
            out_avals=tuple(out_avals),
            in_names=tuple(all_in_names),
            out_names=tuple(out_names),
            lowering_input_output_aliases=(),
            sim_require_finite=True,
            sim_require_nnan=True,
            nc=nc,
        )
        return tuple(outs)

    devices = jax.devices()[:N_CORES]
    mesh = Mesh(np.asarray(devices), ("core",))
    in_specs = (PartitionSpec("core"),) * (n_params + n_outs)
    out_specs = (PartitionSpec("core"),) * n_outs
    sharded = jax.jit(
        shard_map(_body, mesh=mesh, in_specs=in_specs, out_specs=out_specs,
                  check_rep=False),
        donate_argnums=donate, keep_unused=True)

    def runner(in_maps):
        concat_in = [
            np.concatenate([np.asarray(m[name]) for m in in_maps], axis=0)
            for name in in_names
        ]
        concat_zeros = [
            np.zeros((N_CORES * z.shape[0], *z.shape[1:]), z.dtype)
            for z in zero_outs
        ]
        out_arrs = sharded(*concat_in, *concat_zeros)
        return [
            {name: np.asarray(out_arrs[i]).reshape(N_CORES, *out_avals[i].shape)[c]
             for i, name in enumerate(out_names)}
            for c in range(N_CORES)
        ]

    _CACHE["runner"] = runner
    _CACHE["nc"] = nc
    return runner


def make_in_maps(x, qkv_w, proj_w, proj_b):
    bf16 = ml_dtypes.bfloat16
    qkv_w = np.ascontiguousarray(np.asarray(qkv_w, dtype=np.float32))
    # reorder Q|K columns into per-head-pair [Q_hp(128) | K_hp(128)] blocks
    wqk_i = np.empty((C, 2 * C), dtype=np.float32)
    for hp in range(CT):
        wqk_i[:, hp * 256:hp * 256 + 128] = qkv_w[:, hp * 128:(hp + 1) * 128]
        wqk_i[:, hp * 256 + 128:(hp + 1) * 256] = \
            qkv_w[:, C + hp * 128:C + (hp + 1) * 128]
    wqk_i = wqk_i.astype(bf16)
    wv = qkv_w[:, 2 * C:3 * C].astype(bf16)
    pw_b = np.asarray(proj_w, dtype=np.float32).astype(bf16)
    pb = np.asarray(proj_b, dtype=np.float32).reshape(1, C)
    return [
        {
            "x_t": np.ascontiguousarray(
                np.asarray(x[b], dtype=np.float32).T).astype(bf16),
            "wqk": wqk_i,
            "wv": wv,
            "pw": pw_b,
            "pb": pb,
        }
        for b in range(N_CORES)
    ]


def kernel(x, qkv_w, proj_w, proj_b):
    runner = _get_runner()
    results = runner(make_in_maps(x, qkv_w, proj_w, proj_b))
    return np.stack([results[b]["out"] for b in range(N_CORES)], axis=0)


# revision 14
# speedup vs baseline: 1.4369x; 1.3327x over previous
"""Multi-head attention block (QKV proj + softmax attention + out proj) on 8
Trainium2 NeuronCores, data-parallel over the batch dimension (one batch
element per core).

Self-contained: hardcodes shapes for x [8, 1024, 768], qkv_w [768, 2304],
proj_w [768, 768], proj_b [768]; returns [8, 1024, 768] float32.

v2 design notes (why this is structured the way it is):
- bf16 matmul operands everywhere: weight loads use the background weight
  buffer / FWL so LDWEIGHTS hides under the matmul stream (f32r serializes
  ~180ns/MM), and input DMA bytes halve.
- The exp() of the 12.6M attention scores runs only on ScalarE
  ((N+352)/1.2ns per activate => ~110us total). That is co-bottleneck with
  the PE (~123us), so the QKV matmuls for head-pair hp+1 are emitted
  interleaved with the scores/AV of hp: ScalarE starts exping ~30us into
  the kernel and never starves.
- Softmax denominator comes for free as a 65th V_AUG row; the normalize is
  DVE reciprocal + GpSimd partition_broadcast + GpSimd multiply (the old
  DRAM-roundtrip broadcast exposed ~15us after the last head-pair).
- Scores matmuls use K=64 row tiles (head A rows 0-63, head B 64-127)
  emitted adjacently so both heads stream through the PE concurrently.
"""

import numpy as np
import ml_dtypes

import concourse.bass as bass
import concourse.mybir as mybir
import concourse.tile as tile
from concourse import bacc

N_CORES = 8
N = 1024          # tokens per batch element
C = 768           # model dim
H = 12            # heads
HD = 64           # head dim
CT = C // 128     # 6 contraction tiles
TT = N // 128     # 8 token tiles
SCALE = HD ** -0.5

F32 = mybir.dt.float32
BF16 = mybir.dt.bfloat16


def _build():
    nc = bacc.Bacc("TRN2", target_bir_lowering=False, debug=False,
                   num_devices=N_CORES)
    x_t = nc.dram_tensor("x_t", [C, N], BF16, kind="ExternalInput").ap()
    # wqk columns host-reordered: per head-pair hp, cols [hp*256, hp*256+128)
    # are Q features, [hp*256+128, (hp+1)*256) are K features.
    wqk = nc.dram_tensor("wqk", [C, 2 * C], BF16, kind="ExternalInput").ap()
    wv = nc.dram_tensor("wv", [C, C], BF16, kind="ExternalInput").ap()
    pw = nc.dram_tensor("pw", [C, C], BF16, kind="ExternalInput").ap()
    pb = nc.dram_tensor("pb", [1, C], F32, kind="ExternalInput").ap()
    out = nc.dram_tensor("out", [N, C], F32, kind="ExternalOutput").ap()

    with tile.TileContext(nc) as tc:
        _emit(nc, tc, x_t, wqk, wv, pw, pb, out)
    nc.compile()
    return nc


def _emit(nc, tc, x_t, wqk, wv, pw, pb, out):
    from contextlib import ExitStack
    ctx = ExitStack()
    with ctx:
        xt_pool = ctx.enter_context(tc.tile_pool(name="xt", bufs=1))
        wqk_pool = ctx.enter_context(tc.tile_pool(name="wqk", bufs=1))
        wv_pool = ctx.enter_context(tc.tile_pool(name="wv", bufs=1))
        pw_pool = ctx.enter_context(tc.tile_pool(name="pw", bufs=1))
        qk_pool = ctx.enter_context(tc.tile_pool(name="qk", bufs=1))
        vaug_pool = ctx.enter_context(tc.tile_pool(name="vaug", bufs=1))
        outt_pool = ctx.enter_context(tc.tile_pool(name="outt", bufs=1))
        exps_pool = ctx.enter_context(tc.tile_pool(name="exps", bufs=14))
        nrm_pool = ctx.enter_context(tc.tile_pool(name="nrm", bufs=2))
        rcp_pool = ctx.enter_context(tc.tile_pool(name="rcp", bufs=2))
        bc_pool = ctx.enter_context(tc.tile_pool(name="bc", bufs=2))
        const_pool = ctx.enter_context(tc.tile_pool(name="const", bufs=1))
        outsb_pool = ctx.enter_context(tc.tile_pool(name="outsb", bufs=2))
        dram_pool = ctx.enter_context(tc.tile_pool(name="drs", bufs=4, space="DRAM"))

        # ---- input DMAs, spread over idle engine queues, large packets ----
        # Wv first (gates the V matmuls), per-ct blocks: 1.5KB packets.
        WV = wv_pool.tile([128, CT, C], BF16, tag="wv")
        for ct in range(CT):
            nc.sync.dma_start(WV[:, ct, :], wv[ct * 128:(ct + 1) * 128, :])
        # XT in two token halves on the scalar queue (runs parallel to Wv;
        # scalar's exp work only starts much later).
        XT = xt_pool.tile([128, CT, N], BF16, tag="xt")
        for h in range(2):
            nc.scalar.dma_start(
                XT[:, :, h * 512:(h + 1) * 512],
                x_t[:, h * 512:(h + 1) * 512].rearrange("(c p) n -> p c n", p=128))
        # WQK per head-pair-pair blocks (512 cols = 1KB packets), gpsimd queue.
        WQK = wqk_pool.tile([128, CT, 2 * C], BF16, tag="wqk")
        for g in range(3):
            nc.gpsimd.dma_start(
                WQK[:, :, g * 512:(g + 1) * 512],
                wqk[:, g * 512:(g + 1) * 512].rearrange("(c p) f -> p c f", p=128))
        # proj weights + bias (needed only ~100us in).
        PW = pw_pool.tile([128, CT, C], BF16, tag="pw")
        nc.scalar.dma_start(PW[:], pw.rearrange("(c p) f -> p c f", p=128))
        pbb = const_pool.tile([128, C], F32, tag="pb")
        pb_src = pb[:, :]
        pb_bcast = bass.AP(tensor=pb_src.tensor, offset=pb_src.offset,
                           ap=[[0, 128]] + [list(a) for a in pb_src.ap[1:]])
        nc.gpsimd.dma_start(pbb[:], pb_bcast)

        ones_bf = const_pool.tile([128, 96], BF16, tag="ones")
        nc.vector.memset(ones_bf[:], 1.0)
        V_AUG = vaug_pool.tile([128, TT, H, HD + 1], BF16, tag="vaug")
        nc.vector.tensor_copy(
            V_AUG[:, :, :, HD:HD + 1].rearrange("p t h one -> p (t h one)"),
            ones_bf[:])

        QT = qk_pool.tile([128, CT, N], BF16, tag="qt")
        KT = qk_pool.tile([128, CT, N], BF16, tag="kt")
        outT = outt_pool.tile([128, CT, N], BF16, tag="outt")

        sc_ps = ctx.enter_context(tc.tile_pool(name="scps", bufs=2, space="PSUM"))
        av_ps = ctx.enter_context(tc.tile_pool(name="avps", bufs=2, space="PSUM"))

        # PE warmup: ~16 dummy matmuls on a constant tile while the input
        # DMAs stream in. Sustained PE activity flips the HAM clock gate to
        # K=8/8 (~3.4us busy window) so the first real matmuls run at
        # 2.4GHz instead of 1.2GHz.
        warm = const_pool.tile([128, 512], BF16, tag="warm")
        nc.vector.memset(warm[:], 0.5)
        wps = av_ps.tile([128, 1024], F32, tag="av")
        for _ in range(16):
            nc.tensor.matmul(wps[:, 0:512], lhsT=warm[:, 0:128],
                             rhs=warm[:], start=True, stop=True)

        # ---- emit helpers ----
        def emit_v(tt):
            # V matmuls use the av PSUM pool: keeps the scores pool rotation
            # free of V-eviction dependencies, and the av slots aren't needed
            # until the first AV accumulation (after all V tiles retire).
            ps = av_ps.tile([128, 1024], F32, tag="av")
            for w0, wn in ((0, 512), (512, 256)):
                for ct in range(CT):
                    nc.tensor.matmul(
                        ps[:, w0:w0 + wn],
                        lhsT=XT[:, ct, tt * 128:(tt + 1) * 128],
                        rhs=WV[:, ct, w0:w0 + wn],
                        start=(ct == 0), stop=(ct == CT - 1))
            nc.vector.tensor_copy(
                V_AUG[:, tt, :, 0:HD],
                ps[:, :C].rearrange("p (h d) -> p h d", d=HD))

        def emit_qk(hp, which):
            # which: 0 => Q chunk of head-pair hp, 1 => K chunk
            dest = QT if which == 0 else KT
            f0 = hp * 256 + which * 128
            ps = sc_ps.tile([128, 1024], F32, tag="sc")
            for ct in range(CT):
                for qc in range(2):
                    nc.tensor.matmul(
                        ps[:, qc * 512:(qc + 1) * 512],
                        lhsT=WQK[:, ct, f0:f0 + 128],
                        rhs=XT[:, ct, qc * 512:(qc + 1) * 512],
                        start=(ct == 0), stop=(ct == CT - 1))
            nc.vector.tensor_copy(dest[:, hp, :], ps[:])

        def emit_scores(hp, kt):
            psA = sc_ps.tile([128, 1024], F32, tag="sc")
            psB = sc_ps.tile([128, 1024], F32, tag="sc")
            for qc in range(2):
                nc.tensor.matmul(
                    psA[:, qc * 512:(qc + 1) * 512],
                    lhsT=KT[0:64, hp, kt * 128:(kt + 1) * 128],
                    rhs=QT[0:64, hp, qc * 512:(qc + 1) * 512],
                    start=True, stop=True)
                nc.tensor.matmul(
                    psB[:, qc * 512:(qc + 1) * 512],
                    lhsT=KT[64:128, hp, kt * 128:(kt + 1) * 128],
                    rhs=QT[64:128, hp, qc * 512:(qc + 1) * 512],
                    start=True, stop=True)
            eA = exps_pool.tile([128, N], BF16, tag="exps")
            eB = exps_pool.tile([128, N], BF16, tag="exps")
            nc.scalar.activation(eA[:], psA[:], mybir.ActivationFunctionType.Exp,
                                 scale=SCALE)
            nc.scalar.activation(eB[:], psB[:], mybir.ActivationFunctionType.Exp,
                                 scale=SCALE)
            return eA, eB

        def emit_av(hp, kt, eA, eB, avA, avB):
            for av, e, head in ((avA, eA, 2 * hp), (avB, eB, 2 * hp + 1)):
                for qc in range(2):
                    nc.tensor.matmul(
                        av[:, qc * 512:(qc + 1) * 512],
                        lhsT=V_AUG[:, kt, head, :],
                        rhs=e[:, qc * 512:(qc + 1) * 512],
                        start=(kt == 0), stop=(kt == TT - 1))

        def emit_normalize(hp, avA, avB):
            # Denominator reciprocal via the DRAM-roundtrip reshape
            # ([1,1024] -> [64,16] so the DVE reciprocal runs 64 lanes wide)
            # plus a stride-0 DRAM->SBUF broadcast: all ops in this chain are
            # HW-proven. The chain (~4.5us latency) rides the idle gpsimd
            # queue and only the last head-pair exposes it.
            Us, bcs = [], []
            for av in (avA, avB):
                U = nrm_pool.tile([HD + 1, N], F32, tag="U")
                nc.vector.tensor_copy(U[:], av[:])
                dscr = dram_pool.tile([N], F32, tag="dscr")
                nc.gpsimd.dma_start(dscr[:], U[HD:HD + 1, :])
                Dt = rcp_pool.tile([64, 16], F32, tag="Dt")
                nc.gpsimd.dma_start(Dt[:], dscr[:].rearrange("(p j) -> p j", j=16))
                Rt = rcp_pool.tile([64, 16], F32, tag="Rt")
                scr = rcp_pool.tile([64, 16], F32, tag="scr")
                nc.vector.reciprocal_approx_accurate(Rt[:], Dt[:], scr[:])
                rscr = dram_pool.tile([N], F32, tag="rscr")
                nc.gpsimd.dma_start(rscr[:].rearrange("(p j) -> p j", j=16), Rt[:])
                bcst = bc_pool.tile([64, N], F32, tag="bc")
                rs = rscr[:]
                bcast_ap = bass.AP(tensor=rs.tensor, offset=rs.offset,
                                   ap=[[0, 64]] + [list(a) for a in rs.ap])
                nc.gpsimd.dma_start(bcst[:], bcast_ap)
                Us.append(U)
                bcs.append(bcst)
            for U, bcst, poff in ((Us[0], bcs[0], 0), (Us[1], bcs[1], 64)):
                nc.vector.tensor_mul(
                    outT[poff:poff + 64, hp, :], U[0:HD, :], bcst[:])

        def emit_proj(tt):
            ps = sc_ps.tile([128, 1024], F32, tag="sc")
            for ct in range(CT):
                for nch in range(2):
                    nc.tensor.matmul(
                        ps[:, nch * 512:nch * 512 + 384],
                        lhsT=outT[:, ct, tt * 128:(tt + 1) * 128],
                        rhs=PW[:, ct, nch * 384:(nch + 1) * 384],
                        start=(ct == 0), stop=(ct == CT - 1))
            osb = outsb_pool.tile([128, C], F32, tag="outsb")
            ps_v = ps[:].rearrange("p (c x) -> p c x", c=2)[:, :, 0:384]
            osb_v = osb[:].rearrange("p (c x) -> p c x", c=2)
            pbb_v = pbb[:].rearrange("p (c x) -> p c x", c=2)
            nc.vector.tensor_add(osb_v, ps_v, pbb_v)
            nc.sync.dma_start(out[tt * 128:(tt + 1) * 128, :], osb[:])

        # ---- emission schedule ----
        # QK for head-pair 0 first so ScalarE's exp stream starts ~18us in;
        # the V matmuls interleave into hp0's score iterations (kts 0-3).
        # AV for hp0 is held back (pend>=5) until all V tiles are emitted,
        # else the AV matmul would head-block the PE FIFO on V_AUG writes
        # queued behind it.
        emit_qk(0, 0)
        emit_qk(0, 1)
        for hp in range(CT):
            avA = avB = None
            pend = []
            thresh = 5 if hp == 0 else 3
            for kt in range(TT):
                eA, eB = emit_scores(hp, kt)
                pend.append((hp, kt, eA, eB))
                if hp == 0:
                    if kt < 4:
                        emit_v(2 * kt)
                        emit_v(2 * kt + 1)
                    if kt == 4 and hp + 1 < CT:
                        emit_qk(hp + 1, 0)
                    if kt == 6 and hp + 1 < CT:
                        emit_qk(hp + 1, 1)
                else:
                    if kt == 1 and hp + 1 < CT:
                        emit_qk(hp + 1, 0)
                    if kt == 6 and hp + 1 < CT:
                        # at kt6 so the QK matmuls give the PE work across the
                        # head-pair boundary while the last exps drain
                        emit_qk(hp + 1, 1)
                if len(pend) >= thresh:
                    if avA is None:
                        avA = av_ps.tile([HD + 1, 1024], F32, tag="av")
                        avB = av_ps.tile([HD + 1, 1024], F32, tag="av")
                    emit_av(*pend.pop(0), avA, avB)
            if avA is None:
                avA = av_ps.tile([HD + 1, 1024], F32, tag="av")
                avB = av_ps.tile([HD + 1, 1024], F32, tag="av")
            for args in pend:
                emit_av(*args, avA, avB)
            emit_normalize(hp, avA, avB)
        for tt in range(TT):
            emit_proj(tt)


_CACHE = {}


def _get_runner():
    """Build + compile once; return a callable(in_maps) -> list of out dicts.

    Keeps a persistent jitted shard_map executable so repeat calls skip
    retracing/recompiling (mirrors bass2jax.run_bass_via_pjrt).
    """
    if "runner" in _CACHE:
        return _CACHE["runner"]

    import jax
    from jax.experimental.shard_map import shard_map
    from jax.sharding import Mesh, PartitionSpec
    from concourse import bass2jax

    nc = _build()
    bass2jax.install_neuronx_cc_hook()

    partition_name = (nc.partition_id_tensor.name if nc.partition_id_tensor
                      else None)
    in_names, out_names, out_avals, zero_outs = [], [], [], []
    for alloc in nc.m.functions[0].allocations:
        if not isinstance(alloc, mybir.MemoryLocationSet):
            continue
        name = alloc.memorylocations[0].name
        if alloc.kind == "ExternalInput":
            if name != partition_name:
                in_names.append(name)
        elif alloc.kind == "ExternalOutput":
            out_names.append(name)
            shape = tuple(alloc.tensor_shape)
            dtype = mybir.dt.np(alloc.dtype)
            out_avals.append(jax.core.ShapedArray(shape, dtype))
            zero_outs.append(np.zeros(shape, dtype))
    n_params = len(in_names)
    n_outs = len(out_avals)
    all_in_names = list(in_names) + list(out_names)
    if partition_name is not None:
        all_in_names.append(partition_name)
    donate = tuple(range(n_params, n_params + n_outs))

    def _body(*args):
        operands = list(args)
        if partition_name is not None:
            operands.append(bass2jax.partition_id_tensor())
        outs = bass2jax._bass_exec_p.bind(
            *operands,
            out_avals=tuple(out_avals),
            in_names=tuple(all_in_names),
            out_names=tuple(out_names),
            lowering_input_output_aliases=(),
            sim_require_finite=True,
            sim_require_nnan=True,
            nc=nc,
        )
        return tuple(outs)

    devices = jax.devices()[:N_CORES]
    mesh = Mesh(np.asarray(devices), ("core",))
    in_specs = (PartitionSpec("core"),) * (n_params + n_outs)
    out_specs = (PartitionSpec("core"),) * n_outs
    sharded = jax.jit(
        shard_map(_body, mesh=mesh, in_specs=in_specs, out_specs=out_specs,
                  check_rep=False),
        donate_argnums=donate, keep_unused=True)

    def runner(in_maps):
        concat_in = [
            np.concatenate([np.asarray(m[name]) for m in in_maps], axis=0)
            for name in in_names
        ]
        concat_zeros = [
            np.zeros((N_CORES * z.shape[0], *z.shape[1:]), z.dtype)
            for z in zero_outs
        ]
        out_arrs = sharded(*concat_in, *concat_zeros)
        return [
            {name: np.asarray(out_arrs[i]).reshape(N_CORES, *out_avals[i].shape)[c]
             for i, name in enumerate(out_names)}
            for c in range(N_CORES)
        ]

    _CACHE["runner"] = runner
    _CACHE["nc"] = nc
    return runner


def make_in_maps(x, qkv_w, proj_w, proj_b):
    bf16 = ml_dtypes.bfloat16
    qkv_w = np.ascontiguousarray(np.asarray(qkv_w, dtype=np.float32))
    # reorder Q|K columns into per-head-pair [Q_hp(128) | K_hp(128)] blocks
    wqk_i = np.empty((C, 2 * C), dtype=np.float32)
    for hp in range(CT):
        wqk_i[:, hp * 256:hp * 256 + 128] = qkv_w[:, hp * 128:(hp + 1) * 128]
        wqk_i[:, hp * 256 + 128:(hp + 1) * 256] = \
            qkv_w[:, C + hp * 128:C + (hp + 1) * 128]
    wqk_i = wqk_i.astype(bf16)
    wv = qkv_w[:, 2 * C:3 * C].astype(bf16)
    pw_b = np.asarray(proj_w, dtype=np.float32).astype(bf16)
    pb = np.asarray(proj_b, dtype=np.float32).reshape(1, C)
    return [
        {
            "x_t": np.ascontiguousarray(
                np.asarray(x[b], dtype=np.float32).T).astype(bf16),
            "wqk": wqk_i,
            "wv": wv,
            "pw": pw_b,
            "pb": pb,
        }
        for b in range(N_CORES)
    ]


def kernel(x, qkv_w, proj_w, proj_b):
    runner = _get_runner()
    results = runner(make_in_maps(x, qkv_w, proj_w, proj_b))
    return np.stack([results[b]["out"] for b in range(N_CORES)], axis=0)


# revision 18
# speedup vs baseline: 1.4648x; 1.0194x over previous
"""Multi-head attention block (QKV proj + softmax attention + out proj) on 8
Trainium2 NeuronCores, data-parallel over the batch dimension (one batch
element per core).

Self-contained: hardcodes shapes for x [8, 1024, 768], qkv_w [768, 2304],
proj_w [768, 768], proj_b [768]; returns [8, 1024, 768] float32.

v2 design notes (why this is structured the way it is):
- bf16 matmul operands everywhere: weight loads use the background weight
  buffer / FWL so LDWEIGHTS hides under the matmul stream (f32r serializes
  ~180ns/MM), and input DMA bytes halve.
- The exp() of the 12.6M attention scores runs only on ScalarE
  ((N+352)/1.2ns per activate => ~110us total). That is co-bottleneck with
  the PE (~123us), so the QKV matmuls for head-pair hp+1 are emitted
  interleaved with the scores/AV of hp: ScalarE starts exping ~30us into
  the kernel and never starves.
- Softmax denominator comes for free as a 65th V_AUG row; the normalize is
  DVE reciprocal + GpSimd partition_broadcast + GpSimd multiply (the old
  DRAM-roundtrip broadcast exposed ~15us after the last head-pair).
- Scores matmuls use K=64 row tiles (head A rows 0-63, head B 64-127)
  emitted adjacently so both heads stream through the PE concurrently.
"""

import numpy as np
import ml_dtypes

import concourse.bass as bass
import concourse.mybir as mybir
import concourse.tile as tile
from concourse import bacc

N_CORES = 8
N = 1024          # tokens per batch element
C = 768           # model dim
H = 12            # heads
HD = 64           # head dim
CT = C // 128     # 6 contraction tiles
TT = N // 128     # 8 token tiles
SCALE = HD ** -0.5

F32 = mybir.dt.float32
BF16 = mybir.dt.bfloat16


def _build():
    nc = bacc.Bacc("TRN2", target_bir_lowering=False, debug=False,
                   num_devices=N_CORES)
    x_t = nc.dram_tensor("x_t", [C, N], BF16, kind="ExternalInput").ap()
    # wqk columns host-reordered: per head-pair hp, cols [hp*256, hp*256+128)
    # are Q features, [hp*256+128, (hp+1)*256) are K features.
    wqk = nc.dram_tensor("wqk", [C, 2 * C], BF16, kind="ExternalInput").ap()
    wv = nc.dram_tensor("wv", [C, C], BF16, kind="ExternalInput").ap()
    pw = nc.dram_tensor("pw", [C, C], BF16, kind="ExternalInput").ap()
    pb = nc.dram_tensor("pb", [1, C], F32, kind="ExternalInput").ap()
    out = nc.dram_tensor("out", [N, C], F32, kind="ExternalOutput").ap()

    with tile.TileContext(nc) as tc:
        _emit(nc, tc, x_t, wqk, wv, pw, pb, out)
    nc.compile()
    return nc


def _emit(nc, tc, x_t, wqk, wv, pw, pb, out):
    from contextlib import ExitStack
    ctx = ExitStack()
    with ctx:
        xt_pool = ctx.enter_context(tc.tile_pool(name="xt", bufs=1))
        wqk_pool = ctx.enter_context(tc.tile_pool(name="wqk", bufs=1))
        wv_pool = ctx.enter_context(tc.tile_pool(name="wv", bufs=1))
        pw_pool = ctx.enter_context(tc.tile_pool(name="pw", bufs=1))
        qk_pool = ctx.enter_context(tc.tile_pool(name="qk", bufs=1))
        vaug_pool = ctx.enter_context(tc.tile_pool(name="vaug", bufs=1))
        outt_pool = ctx.enter_context(tc.tile_pool(name="outt", bufs=1))
        exps_pool = ctx.enter_context(tc.tile_pool(name="exps", bufs=14))
        nrm_pool = ctx.enter_context(tc.tile_pool(name="nrm", bufs=2))
        rcp_pool = ctx.enter_context(tc.tile_pool(name="rcp", bufs=2))
        bc_pool = ctx.enter_context(tc.tile_pool(name="bc", bufs=2))
        const_pool = ctx.enter_context(tc.tile_pool(name="const", bufs=1))
        outsb_pool = ctx.enter_context(tc.tile_pool(name="outsb", bufs=2))
        dram_pool = ctx.enter_context(tc.tile_pool(name="drs", bufs=4, space="DRAM"))

        # ---- input DMAs, spread over idle engine queues, large packets ----
        # Wv first (gates the V matmuls), per-ct blocks: 1.5KB packets.
        WV = wv_pool.tile([128, CT, C], BF16, tag="wv")
        for ct in range(CT):
            nc.sync.dma_start(WV[:, ct, :], wv[ct * 128:(ct + 1) * 128, :])
        # XT in two token halves on the scalar queue (runs parallel to Wv;
        # scalar's exp work only starts much later).
        XT = xt_pool.tile([128, CT, N], BF16, tag="xt")
        for h in range(2):
            nc.scalar.dma_start(
                XT[:, :, h * 512:(h + 1) * 512],
                x_t[:, h * 512:(h + 1) * 512].rearrange("(c p) n -> p c n", p=128))
        # WQK per head-pair-pair blocks (512 cols = 1KB packets), gpsimd queue.
        WQK = wqk_pool.tile([128, CT, 2 * C], BF16, tag="wqk")
        for g in range(3):
            nc.gpsimd.dma_start(
                WQK[:, :, g * 512:(g + 1) * 512],
                wqk[:, g * 512:(g + 1) * 512].rearrange("(c p) f -> p c f", p=128))
        # proj weights + bias (needed only ~100us in).
        PW = pw_pool.tile([128, CT, C], BF16, tag="pw")
        nc.scalar.dma_start(PW[:], pw.rearrange("(c p) f -> p c f", p=128))
        pbb = const_pool.tile([128, C], F32, tag="pb")
        pb_src = pb[:, :]
        pb_bcast = bass.AP(tensor=pb_src.tensor, offset=pb_src.offset,
                           ap=[[0, 128]] + [list(a) for a in pb_src.ap[1:]])
        nc.gpsimd.dma_start(pbb[:], pb_bcast)

        ones_bf = const_pool.tile([128, 96], BF16, tag="ones")
        nc.vector.memset(ones_bf[:], 1.0)
        V_AUG = vaug_pool.tile([128, TT, H, HD + 1], BF16, tag="vaug")
        nc.vector.tensor_copy(
            V_AUG[:, :, :, HD:HD + 1].rearrange("p t h one -> p (t h one)"),
            ones_bf[:])

        QT = qk_pool.tile([128, CT, N], BF16, tag="qt")
        KT = qk_pool.tile([128, CT, N], BF16, tag="kt")
        outT = outt_pool.tile([128, CT, N], BF16, tag="outt")

        sc_ps = ctx.enter_context(tc.tile_pool(name="scps", bufs=2, space="PSUM"))
        av_ps = ctx.enter_context(tc.tile_pool(name="avps", bufs=2, space="PSUM"))

        # PE warmup: ~16 dummy matmuls on a constant tile while the input
        # DMAs stream in. Sustained PE activity flips the HAM clock gate to
        # K=8/8 (~3.4us busy window) so the first real matmuls run at
        # 2.4GHz instead of 1.2GHz.
        warm = const_pool.tile([128, 512], BF16, tag="warm")
        nc.vector.memset(warm[:], 0.5)
        wps = av_ps.tile([128, 1024], F32, tag="av")
        for _ in range(36):
            nc.tensor.matmul(wps[:, 0:512], lhsT=warm[:, 0:128],
                             rhs=warm[:], start=True, stop=True)

        # ---- emit helpers ----
        def emit_v(tt):
            # V matmuls use the av PSUM pool: keeps the scores pool rotation
            # free of V-eviction dependencies, and the av slots aren't needed
            # until the first AV accumulation (after all V tiles retire).
            ps = av_ps.tile([128, 1024], F32, tag="av")
            for w0, wn in ((0, 512), (512, 256)):
                for ct in range(CT):
                    nc.tensor.matmul(
                        ps[:, w0:w0 + wn],
                        lhsT=XT[:, ct, tt * 128:(tt + 1) * 128],
                        rhs=WV[:, ct, w0:w0 + wn],
                        start=(ct == 0), stop=(ct == CT - 1))
            nc.vector.tensor_copy(
                V_AUG[:, tt, :, 0:HD],
                ps[:, :C].rearrange("p (h d) -> p h d", d=HD))

        def emit_qk(hp, which):
            # which: 0 => Q chunk of head-pair hp, 1 => K chunk
            dest = QT if which == 0 else KT
            f0 = hp * 256 + which * 128
            ps = sc_ps.tile([128, 1024], F32, tag="sc")
            # qc-outer: the qc=0 group only needs the first XT token half,
            # so the very first matmuls start ~2.5us earlier at kernel start
            for qc in range(2):
                for ct in range(CT):
                    nc.tensor.matmul(
                        ps[:, qc * 512:(qc + 1) * 512],
                        lhsT=WQK[:, ct, f0:f0 + 128],
                        rhs=XT[:, ct, qc * 512:(qc + 1) * 512],
                        start=(ct == 0), stop=(ct == CT - 1))
            nc.vector.tensor_copy(dest[:, hp, :], ps[:])

        def emit_scores(hp, kt):
            psA = sc_ps.tile([128, 1024], F32, tag="sc")
            psB = sc_ps.tile([128, 1024], F32, tag="sc")
            for qc in range(2):
                nc.tensor.matmul(
                    psA[:, qc * 512:(qc + 1) * 512],
                    lhsT=KT[0:64, hp, kt * 128:(kt + 1) * 128],
                    rhs=QT[0:64, hp, qc * 512:(qc + 1) * 512],
                    start=True, stop=True)
                nc.tensor.matmul(
                    psB[:, qc * 512:(qc + 1) * 512],
                    lhsT=KT[64:128, hp, kt * 128:(kt + 1) * 128],
                    rhs=QT[64:128, hp, qc * 512:(qc + 1) * 512],
                    start=True, stop=True)
            eA = exps_pool.tile([128, N], BF16, tag="exps")
            eB = exps_pool.tile([128, N], BF16, tag="exps")
            nc.scalar.activation(eA[:], psA[:], mybir.ActivationFunctionType.Exp,
                                 scale=SCALE)
            nc.scalar.activation(eB[:], psB[:], mybir.ActivationFunctionType.Exp,
                                 scale=SCALE)
            return eA, eB

        def emit_av(hp, kt, eA, eB, avA, avB):
            for av, e, head in ((avA, eA, 2 * hp), (avB, eB, 2 * hp + 1)):
                for qc in range(2):
                    nc.tensor.matmul(
                        av[:, qc * 512:(qc + 1) * 512],
                        lhsT=V_AUG[:, kt, head, :],
                        rhs=e[:, qc * 512:(qc + 1) * 512],
                        start=(kt == 0), stop=(kt == TT - 1))

        def emit_normalize(hp, avA, avB):
            # Denominator reciprocal via the DRAM-roundtrip reshape
            # ([1,1024] -> [64,16] so the DVE reciprocal runs 64 lanes wide)
            # plus a stride-0 DRAM->SBUF broadcast: all ops in this chain are
            # HW-proven. The chain (~4.5us latency) rides the idle gpsimd
            # queue and only the last head-pair exposes it.
            Us, Rts, bcs = [], [], []
            for av in (avA, avB):
                U = nrm_pool.tile([HD + 1, N], F32, tag="U")
                nc.vector.tensor_copy(U[:], av[:])
                dscr = dram_pool.tile([N], F32, tag="dscr")
                nc.sync.dma_start(dscr[:], U[HD:HD + 1, :])
                Dt = rcp_pool.tile([64, 16], F32, tag="Dt")
                nc.sync.dma_start(Dt[:], dscr[:].rearrange("(p j) -> p j", j=16))
                Us.append((U, Dt))
            for U, Dt in Us:
                Rt = rcp_pool.tile([64, 16], F32, tag="Rt")
                scr = rcp_pool.tile([64, 16], F32, tag="scr")
                nc.vector.reciprocal_approx_accurate(Rt[:], Dt[:], scr[:])
                Rts.append(Rt)
            for Rt in Rts:
                rscr = dram_pool.tile([N], F32, tag="rscr")
                nc.sync.dma_start(rscr[:].rearrange("(p j) -> p j", j=16), Rt[:])
                bcst = bc_pool.tile([64, N], F32, tag="bc")
                rs = rscr[:]
                bcast_ap = bass.AP(tensor=rs.tensor, offset=rs.offset,
                                   ap=[[0, 64]] + [list(a) for a in rs.ap])
                nc.sync.dma_start(bcst[:], bcast_ap)
                bcs.append(bcst)
            for (U, _), bcst, poff in ((Us[0], bcs[0], 0), (Us[1], bcs[1], 64)):
                nc.vector.tensor_mul(
                    outT[poff:poff + 64, hp, :], U[0:HD, :], bcst[:])

        def emit_proj(tt):
            ps = sc_ps.tile([128, 1024], F32, tag="sc")
            for ct in range(CT):
                for nch in range(2):
                    nc.tensor.matmul(
                        ps[:, nch * 512:nch * 512 + 384],
                        lhsT=outT[:, ct, tt * 128:(tt + 1) * 128],
                        rhs=PW[:, ct, nch * 384:(nch + 1) * 384],
                        start=(ct == 0), stop=(ct == CT - 1))
            osb = outsb_pool.tile([128, C], F32, tag="outsb")
            ps_v = ps[:].rearrange("p (c x) -> p c x", c=2)[:, :, 0:384]
            osb_v = osb[:].rearrange("p (c x) -> p c x", c=2)
            pbb_v = pbb[:].rearrange("p (c x) -> p c x", c=2)
            nc.vector.tensor_add(osb_v, ps_v, pbb_v)
            nc.sync.dma_start(out[tt * 128:(tt + 1) * 128, :], osb[:])

        # ---- emission schedule ----
        # QK for head-pair 0 first so ScalarE's exp stream starts ~18us in;
        # the V matmuls interleave into hp0's score iterations (kts 0-3).
        # AV for hp0 is held back (pend>=5) until all V tiles are emitted,
        # else the AV matmul would head-block the PE FIFO on V_AUG writes
        # queued behind it.
        emit_qk(0, 0)
        emit_qk(0, 1)
        for hp in range(CT):
            avA = avB = None
            pend = []
            thresh = 5 if hp == 0 else 2
            for kt in range(TT):
                eA, eB = emit_scores(hp, kt)
                pend.append((hp, kt, eA, eB))
                if hp == 0:
                    if kt < 4:
                        emit_v(2 * kt)
                        emit_v(2 * kt + 1)
                    if kt == 4 and hp + 1 < CT:
                        emit_qk(hp + 1, 0)
                    if kt == 6 and hp + 1 < CT:
                        emit_qk(hp + 1, 1)
                else:
                    if kt == 1 and hp + 1 < CT:
                        emit_qk(hp + 1, 0)
                    if kt == 6 and hp + 1 < CT:
                        # at kt6 so the QK matmuls give the PE work across the
                        # head-pair boundary while the last exps drain
                        emit_qk(hp + 1, 1)
                if len(pend) >= thresh:
                    if avA is None:
                        avA = av_ps.tile([HD + 1, 1024], F32, tag="av")
                        avB = av_ps.tile([HD + 1, 1024], F32, tag="av")
                    emit_av(*pend.pop(0), avA, avB)
            if avA is None:
                avA = av_ps.tile([HD + 1, 1024], F32, tag="av")
                avB = av_ps.tile([HD + 1, 1024], F32, tag="av")
            for args in pend:
                emit_av(*args, avA, avB)
            emit_normalize(hp, avA, avB)
        for tt in range(TT):
            emit_proj(tt)


_CACHE = {}


def _get_runner():
    """Build + compile once; return a callable(in_maps) -> list of out dicts.

    Keeps a persistent jitted shard_map executable so repeat calls skip
    retracing/recompiling (mirrors bass2jax.run_bass_via_pjrt).
    """
    if "runner" in _CACHE:
        return _CACHE["runner"]

    import jax
    from jax.experimental.shard_map import shard_map
    from jax.sharding import Mesh, PartitionSpec
    from concourse import bass2jax

    nc = _build()
    bass2jax.install_neuronx_cc_hook()

    partition_name = (nc.partition_id_tensor.name if nc.partition_id_tensor
                      else None)
    in_names, out_names, out_avals, zero_outs = [], [], [], []
    for alloc in nc.m.functions[0].allocations:
        if not isinstance(alloc, mybir.MemoryLocationSet):
            continue
        name = alloc.memorylocations[0].name
        if alloc.kind == "ExternalInput":
            if name != partition_name:
                in_names.append(name)
        elif alloc.kind == "ExternalOutput":
            out_names.append(name)
            shape = tuple(alloc.tensor_shape)
            dtype = mybir.dt.np(alloc.dtype)
            out_avals.append(jax.core.ShapedArray(shape, dtype))
            zero_outs.append(np.zeros(shape, dtype))
    n_params = len(in_names)
    n_outs = len(out_avals)
    all_in_names = list(in_names) + list(out_names)
    if partition_name is not None:
        all_in_names.append(partition_name)
    donate = tuple(range(n_params, n_params + n_outs))

    def _body(*args):
        operands = list(args)
        if partition_name is not None:
            operands.append(bass2jax.partition_id_tensor())
        outs = bass2jax._bass_exec_p.bind(
            *operands,
            out_avals=tuple(out_avals),
            in_names=tuple(all_in_names),
            out_names=tuple(out_names),
            lowering_input_output_aliases=(),
            sim_require_finite=True,
            sim_require_nnan=True,
            nc=nc,
        )
        return tuple(outs)

    devices = jax.devices()[:N_CORES]
    mesh = Mesh(np.asarray(devices), ("core",))
    in_specs = (PartitionSpec("core"),) * (n_params + n_outs)
    out_specs = (PartitionSpec("core"),) * n_outs
    sharded = jax.jit(
        shard_map(_body, mesh=mesh, in_specs=in_specs, out_specs=out_specs,
                  check_rep=False),
        donate_argnums=donate, keep_unused=True)

    def runner(in_maps):
        concat_in = [
            np.concatenate([np.asarray(m[name]) for m in in_maps], axis=0)
            for name in in_names
        ]
        concat_zeros = [
            np.zeros((N_CORES * z.shape[0], *z.shape[1:]), z.dtype)
            for z in zero_outs
        ]
        out_arrs = sharded(*concat_in, *concat_zeros)
        return [
            {name: np.asarray(out_arrs[i]).reshape(N_CORES, *out_avals[i].shape)[c]
             for i, name in enumerate(out_names)}
            for c in range(N_CORES)
        ]

    _CACHE["runner"] = runner
    _CACHE["nc"] = nc
    return runner


def make_in_maps(x, qkv_w, proj_w, proj_b):
    bf16 = ml_dtypes.bfloat16
    qkv_w = np.ascontiguousarray(np.asarray(qkv_w, dtype=np.float32))
    # reorder Q|K columns into per-head-pair [Q_hp(128) | K_hp(128)] blocks
    wqk_i = np.empty((C, 2 * C), dtype=np.float32)
    for hp in range(CT):
        wqk_i[:, hp * 256:hp * 256 + 128] = qkv_w[:, hp * 128:(hp + 1) * 128]
        wqk_i[:, hp * 256 + 128:(hp + 1) * 256] = \
            qkv_w[:, C + hp * 128:C + (hp + 1) * 128]
    wqk_i = wqk_i.astype(bf16)
    wv = qkv_w[:, 2 * C:3 * C].astype(bf16)
    pw_b = np.asarray(proj_w, dtype=np.float32).astype(bf16)
    pb = np.asarray(proj_b, dtype=np.float32).reshape(1, C)
    return [
        {
            "x_t": np.ascontiguousarray(
                np.asarray(x[b], dtype=np.float32).T).astype(bf16),
            "wqk": wqk_i,
            "wv": wv,
            "pw": pw_b,
            "pb": pb,
        }
        for b in range(N_CORES)
    ]


def kernel(x, qkv_w, proj_w, proj_b):
    runner = _get_runner()
    results = runner(make_in_maps(x, qkv_w, proj_w, proj_b))
    return np.stack([results[b]["out"] for b in range(N_CORES)], axis=0)


# revision 25
# speedup vs baseline: 1.5093x; 1.0303x over previous
"""Multi-head attention block (QKV proj + softmax attention + out proj) on 8
Trainium2 NeuronCores, data-parallel over the batch dimension (one batch
element per core).

Self-contained: hardcodes shapes for x [8, 1024, 768], qkv_w [768, 2304],
proj_w [768, 768], proj_b [768]; returns [8, 1024, 768] float32.

v2 design notes (why this is structured the way it is):
- bf16 matmul operands everywhere: weight loads use the background weight
  buffer / FWL so LDWEIGHTS hides under the matmul stream (f32r serializes
  ~180ns/MM), and input DMA bytes halve.
- The exp() of the 12.6M attention scores runs only on ScalarE
  ((N+352)/1.2ns per activate => ~110us total). That is co-bottleneck with
  the PE (~123us), so the QKV matmuls for head-pair hp+1 are emitted
  interleaved with the scores/AV of hp: ScalarE starts exping ~30us into
  the kernel and never starves.
- Softmax denominator comes for free as a 65th V_AUG row; the normalize is
  DVE reciprocal + GpSimd partition_broadcast + GpSimd multiply (the old
  DRAM-roundtrip broadcast exposed ~15us after the last head-pair).
- Scores matmuls use K=64 row tiles (head A rows 0-63, head B 64-127)
  emitted adjacently so both heads stream through the PE concurrently.
"""

import numpy as np
import ml_dtypes

import concourse.bass as bass
import concourse.mybir as mybir
import concourse.tile as tile
from concourse import bacc

N_CORES = 8
N = 1024          # tokens per batch element
C = 768           # model dim
H = 12            # heads
HD = 64           # head dim
CT = C // 128     # 6 contraction tiles
TT = N // 128     # 8 token tiles
SCALE = HD ** -0.5

F32 = mybir.dt.float32
BF16 = mybir.dt.bfloat16


def _build():
    nc = bacc.Bacc("TRN2", target_bir_lowering=False, debug=False,
                   num_devices=N_CORES)
    x_t = nc.dram_tensor("x_t", [C, N], BF16, kind="ExternalInput").ap()
    # wqk columns host-reordered: per head-pair hp, cols [hp*256, hp*256+128)
    # are Q features, [hp*256+128, (hp+1)*256) are K features.
    wqk = nc.dram_tensor("wqk", [C, 2 * C], BF16, kind="ExternalInput").ap()
    wv = nc.dram_tensor("wv", [C, C], BF16, kind="ExternalInput").ap()
    pw = nc.dram_tensor("pw", [C, C], BF16, kind="ExternalInput").ap()
    pb = nc.dram_tensor("pb", [1, C], mybir.dt.float32r, kind="ExternalInput").ap()
    out = nc.dram_tensor("out", [N, C], F32, kind="ExternalOutput").ap()

    with tile.TileContext(nc) as tc:
        _emit(nc, tc, x_t, wqk, wv, pw, pb, out)
    nc.compile()
    return nc


def _emit(nc, tc, x_t, wqk, wv, pw, pb, out):
    from contextlib import ExitStack
    ctx = ExitStack()
    with ctx:
        xt_pool = ctx.enter_context(tc.tile_pool(name="xt", bufs=1))
        wqk_pool = ctx.enter_context(tc.tile_pool(name="wqk", bufs=1))
        wv_pool = ctx.enter_context(tc.tile_pool(name="wv", bufs=1))
        pw_pool = ctx.enter_context(tc.tile_pool(name="pw", bufs=1))
        qk_pool = ctx.enter_context(tc.tile_pool(name="qk", bufs=1))
        vaug_pool = ctx.enter_context(tc.tile_pool(name="vaug", bufs=1))
        outt_pool = ctx.enter_context(tc.tile_pool(name="outt", bufs=1))
        exps_pool = ctx.enter_context(tc.tile_pool(name="exps", bufs=14))
        nrm_pool = ctx.enter_context(tc.tile_pool(name="nrm", bufs=2))
        rcp_pool = ctx.enter_context(tc.tile_pool(name="rcp", bufs=2))
        bc_pool = ctx.enter_context(tc.tile_pool(name="bc", bufs=2))
        const_pool = ctx.enter_context(tc.tile_pool(name="const", bufs=1))
        outsb_pool = ctx.enter_context(tc.tile_pool(name="outsb", bufs=8))
        dram_pool = ctx.enter_context(tc.tile_pool(name="drs", bufs=4, space="DRAM"))

        # ---- input DMAs ----
        # The three DMA queues together saturate HBM (~360GB/s), so order by
        # need-time: XT h0 + WQK g0 gate the first QK matmuls (~13us), XT h1
        # the qc=1 groups, Wv the V matmuls (~20us); everything else is late.
        XT = xt_pool.tile([128, CT, N], BF16, tag="xt")
        WQK = wqk_pool.tile([128, CT, 2 * C], BF16, tag="wqk")
        WV = wv_pool.tile([128, CT, C], BF16, tag="wv")
        PW = pw_pool.tile([128, CT, C], BF16, tag="pw")

        def _xt_dma(eng, h):
            eng.dma_start(
                XT[:, :, h * 512:(h + 1) * 512],
                x_t[:, h * 512:(h + 1) * 512].rearrange("(c p) n -> p c n", p=128))

        def _wqk_dma(eng, g):
            eng.dma_start(
                WQK[:, :, g * 512:(g + 1) * 512],
                wqk[:, g * 512:(g + 1) * 512].rearrange("(c p) f -> p c f", p=128))

        _xt_dma(nc.scalar, 0)
        _xt_dma(nc.sync, 1)
        _wqk_dma(nc.gpsimd, 0)
        for ct in range(CT):
            nc.scalar.dma_start(WV[:, ct, :], wv[ct * 128:(ct + 1) * 128, :])
        _wqk_dma(nc.sync, 1)
        _wqk_dma(nc.gpsimd, 2)
        nc.gpsimd.dma_start(PW[:], pw.rearrange("(c p) f -> p c f", p=128))
        # bias row + a ones column, f32r, for the bias-init matmul in proj
        pb_sb = const_pool.tile([1, C], mybir.dt.float32r, tag="pbsb")
        nc.gpsimd.dma_start(pb_sb[:], pb[:, :])
        ones_f32 = const_pool.tile([1, 128], F32, tag="onesf32")
        nc.vector.memset(ones_f32[:], 1.0)
        ones_fr = const_pool.tile([1, 128], mybir.dt.float32r, tag="onesfr")
        nc.vector.tensor_copy(ones_fr[:], ones_f32[:])

        ones_bf = const_pool.tile([128, 96], BF16, tag="ones")
        nc.vector.memset(ones_bf[:], 1.0)
        V_AUG = vaug_pool.tile([128, TT, H, HD + 1], BF16, tag="vaug")
        nc.vector.tensor_copy(
            V_AUG[:, :, :, HD:HD + 1].rearrange("p t h one -> p (t h one)"),
            ones_bf[:])

        QT = qk_pool.tile([128, CT, N], BF16, tag="qt")
        KT = qk_pool.tile([128, CT, N], BF16, tag="kt")
        outT = outt_pool.tile([128, CT, N], BF16, tag="outt")

        sc_ps = ctx.enter_context(tc.tile_pool(name="scps", bufs=2, space="PSUM"))
        av_ps = ctx.enter_context(tc.tile_pool(name="avps", bufs=2, space="PSUM"))

        # PE warmup: ~16 dummy matmuls on a constant tile while the input
        # DMAs stream in. Sustained PE activity flips the HAM clock gate to
        # K=8/8 (~3.4us busy window) so the first real matmuls run at
        # 2.4GHz instead of 1.2GHz.
        warm = const_pool.tile([128, 512], BF16, tag="warm")
        nc.vector.memset(warm[:], 0.5)
        wps = av_ps.tile([128, 1024], F32, tag="av")
        for _ in range(36):
            nc.tensor.matmul(wps[:, 0:512], lhsT=warm[:, 0:128],
                             rhs=warm[:], start=True, stop=True)

        # ---- emit helpers ----
        def emit_v(tt):
            # V matmuls use the av PSUM pool: keeps the scores pool rotation
            # free of V-eviction dependencies, and the av slots aren't needed
            # until the first AV accumulation (after all V tiles retire).
            ps = av_ps.tile([128, 1024], F32, tag="av")
            for w0, wn in ((0, 512), (512, 256)):
                for ct in range(CT):
                    nc.tensor.matmul(
                        ps[:, w0:w0 + wn],
                        lhsT=XT[:, ct, tt * 128:(tt + 1) * 128],
                        rhs=WV[:, ct, w0:w0 + wn],
                        start=(ct == 0), stop=(ct == CT - 1))
            nc.vector.tensor_copy(
                V_AUG[:, tt, :, 0:HD],
                ps[:, :C].rearrange("p (h d) -> p h d", d=HD))

        def emit_qk(hp, which):
            # which: 0 => Q chunk of head-pair hp, 1 => K chunk
            dest = QT if which == 0 else KT
            f0 = hp * 256 + which * 128
            ps = sc_ps.tile([128, 1024], F32, tag="sc")
            # qc-outer: the qc=0 group only needs the first XT token half,
            # so the very first matmuls start ~2.5us earlier at kernel start
            for qc in range(2):
                for ct in range(CT):
                    nc.tensor.matmul(
                        ps[:, qc * 512:(qc + 1) * 512],
                        lhsT=WQK[:, ct, f0:f0 + 128],
                        rhs=XT[:, ct, qc * 512:(qc + 1) * 512],
                        start=(ct == 0), stop=(ct == CT - 1))
            nc.vector.tensor_copy(dest[:, hp, :], ps[:])

        def emit_scores(hp, kt):
            psA = sc_ps.tile([128, 1024], F32, tag="sc")
            psB = sc_ps.tile([128, 1024], F32, tag="sc")
            for qc in range(2):
                nc.tensor.matmul(
                    psA[:, qc * 512:(qc + 1) * 512],
                    lhsT=KT[0:64, hp, kt * 128:(kt + 1) * 128],
                    rhs=QT[0:64, hp, qc * 512:(qc + 1) * 512],
                    start=True, stop=True)
                nc.tensor.matmul(
                    psB[:, qc * 512:(qc + 1) * 512],
                    lhsT=KT[64:128, hp, kt * 128:(kt + 1) * 128],
                    rhs=QT[64:128, hp, qc * 512:(qc + 1) * 512],
                    start=True, stop=True)
            eA = exps_pool.tile([128, N], BF16, tag="exps")
            eB = exps_pool.tile([128, N], BF16, tag="exps")
            nc.scalar.activation(eA[:], psA[:], mybir.ActivationFunctionType.Exp,
                                 scale=SCALE)
            nc.scalar.activation(eB[:], psB[:], mybir.ActivationFunctionType.Exp,
                                 scale=SCALE)
            return eA, eB

        def emit_av(hp, kt, eA, eB, avA, avB):
            for av, e, head in ((avA, eA, 2 * hp), (avB, eB, 2 * hp + 1)):
                for qc in range(2):
                    nc.tensor.matmul(
                        av[:, qc * 512:(qc + 1) * 512],
                        lhsT=V_AUG[:, kt, head, :],
                        rhs=e[:, qc * 512:(qc + 1) * 512],
                        start=(kt == 0), stop=(kt == TT - 1))

        def emit_normalize_pre(hp, avA, avB):
            # Denominator reciprocal via the DRAM-roundtrip reshape
            # ([1,1024] -> [64,16] so the DVE reciprocal runs 64 lanes wide)
            # plus a stride-0 DRAM->SBUF broadcast: all ops in this chain are
            # HW-proven. The chain (~4.5us latency) rides the idle gpsimd
            # queue and only the last head-pair exposes it.
            Us, Rts, bcs = [], [], []
            for av in (avA, avB):
                U = nrm_pool.tile([HD + 1, N], F32, tag="U")
                nc.vector.tensor_copy(U[:], av[:])
                dscr = dram_pool.tile([N], F32, tag="dscr")
                nc.sync.dma_start(dscr[:], U[HD:HD + 1, :])
                Dt = rcp_pool.tile([64, 16], F32, tag="Dt")
                nc.sync.dma_start(Dt[:], dscr[:].rearrange("(p j) -> p j", j=16))
                Us.append((U, Dt))
            for U, Dt in Us:
                Rt = rcp_pool.tile([64, 16], F32, tag="Rt")
                scr = rcp_pool.tile([64, 16], F32, tag="scr")
                nc.vector.reciprocal_approx_accurate(Rt[:], Dt[:], scr[:])
                Rts.append(Rt)
            for Rt in Rts:
                rscr = dram_pool.tile([N], F32, tag="rscr")
                nc.sync.dma_start(rscr[:].rearrange("(p j) -> p j", j=16), Rt[:])
                bcst = bc_pool.tile([64, N], F32, tag="bc")
                rs = rscr[:]
                bcast_ap = bass.AP(tensor=rs.tensor, offset=rs.offset,
                                   ap=[[0, 64]] + [list(a) for a in rs.ap])
                nc.sync.dma_start(bcst[:], bcast_ap)
                bcs.append(bcst)
            return Us, bcs

        def emit_normalize_muls(hp, Us, bcs):
            for (U, _), bcst, poff in ((Us[0], bcs[0], 0), (Us[1], bcs[1], 64)):
                nc.vector.tensor_mul(
                    outT[poff:poff + 64, hp, :], U[0:HD, :], bcst[:])

        def emit_normalize(hp, avA, avB):
            Us, bcs = emit_normalize_pre(hp, avA, avB)
            emit_normalize_muls(hp, Us, bcs)

        # proj is split: partials (bias + head-pairs 0-4) run while the last
        # head-pair's normalize chain is in flight, with ScalarE (idle after
        # the final exp) doing the PSUM->SBUF evictions; only the ct=5
        # contribution remains after the last normalize multiply.
        def emit_proj_partial(tt):
            ps = sc_ps.tile([128, 1024], F32, tag="sc")
            for nch in range(2):
                nc.tensor.matmul(
                    ps[:, nch * 512:nch * 512 + 384],
                    lhsT=ones_fr[0:1, :],
                    rhs=pb_sb[0:1, nch * 384:(nch + 1) * 384],
                    start=True, stop=False)
            for ct in range(CT - 1):
                for nch in range(2):
                    nc.tensor.matmul(
                        ps[:, nch * 512:nch * 512 + 384],
                        lhsT=outT[:, ct, tt * 128:(tt + 1) * 128],
                        rhs=PW[:, ct, nch * 384:(nch + 1) * 384],
                        start=False, stop=(ct == CT - 2))
            osb = outsb_pool.tile([128, C], F32, tag="outsb")
            ps_v = ps[:].rearrange("p (c x) -> p c x", c=2)[:, :, 0:384]
            osb_v = osb[:].rearrange("p (c x) -> p c x", c=2)
            nc.scalar.activation(osb_v, ps_v, mybir.ActivationFunctionType.Copy)
            return osb

        def emit_proj_final(tt, osb):
            ps = sc_ps.tile([128, 1024], F32, tag="sc")
            for nch in range(2):
                nc.tensor.matmul(
                    ps[:, nch * 512:nch * 512 + 384],
                    lhsT=outT[:, CT - 1, tt * 128:(tt + 1) * 128],
                    rhs=PW[:, CT - 1, nch * 384:(nch + 1) * 384],
                    start=True, stop=True)
            ps_v = ps[:].rearrange("p (c x) -> p c x", c=2)[:, :, 0:384]
            osb_v = osb[:].rearrange("p (c x) -> p c x", c=2)
            nc.vector.tensor_add(osb_v, osb_v, ps_v)
            nc.sync.dma_start(out[tt * 128:(tt + 1) * 128, :], osb[:])

        # ---- emission schedule ----
        # QK for head-pair 0 first so ScalarE's exp stream starts ~18us in;
        # the V matmuls interleave into hp0's score iterations (kts 0-3).
        # AV for hp0 is held back (pend>=5) until all V tiles are emitted,
        # else the AV matmul would head-block the PE FIFO on V_AUG writes
        # queued behind it.
        emit_qk(0, 0)
        emit_qk(0, 1)
        for hp in range(CT):
            avA = avB = None
            pend = []
            thresh = 5 if hp == 0 else 2
            for kt in range(TT):
                eA, eB = emit_scores(hp, kt)
                pend.append((hp, kt, eA, eB))
                if hp == 0:
                    if kt < 4:
                        emit_v(2 * kt)
                        emit_v(2 * kt + 1)
                    if kt == 4 and hp + 1 < CT:
                        emit_qk(hp + 1, 0)
                    if kt == 6 and hp + 1 < CT:
                        emit_qk(hp + 1, 1)
                else:
                    if kt == 1 and hp + 1 < CT:
                        emit_qk(hp + 1, 0)
                    if kt == 6 and hp + 1 < CT:
                        # at kt6 so the QK matmuls give the PE work across the
                        # head-pair boundary while the last exps drain
                        emit_qk(hp + 1, 1)
                if len(pend) >= thresh:
                    if avA is None:
                        avA = av_ps.tile([HD + 1, 1024], F32, tag="av")
                        avB = av_ps.tile([HD + 1, 1024], F32, tag="av")
                    emit_av(*pend.pop(0), avA, avB)
            if avA is None:
                avA = av_ps.tile([HD + 1, 1024], F32, tag="av")
                avB = av_ps.tile([HD + 1, 1024], F32, tag="av")
            for args in pend:
                emit_av(*args, avA, avB)
            if hp < CT - 1:
                emit_normalize(hp, avA, avB)
            else:
                Us, bcs = emit_normalize_pre(hp, avA, avB)
                osbs = [emit_proj_partial(tt) for tt in range(TT)]
                emit_normalize_muls(hp, Us, bcs)
                for tt in range(TT):
                    emit_proj_final(tt, osbs[tt])


_CACHE = {}


def _get_runner():
    """Build + compile once; return a callable(in_maps) -> list of out dicts.

    Keeps a persistent jitted shard_map executable so repeat calls skip
    retracing/recompiling (mirrors bass2jax.run_bass_via_pjrt).
    """
    if "runner" in _CACHE:
        return _CACHE["runner"]

    import jax
    from jax.experimental.shard_map import shard_map
    from jax.sharding import Mesh, PartitionSpec
    from concourse import bass2jax

    nc = _build()
    bass2jax.install_neuronx_cc_hook()

    partition_name = (nc.partition_id_tensor.name if nc.partition_id_tensor
                      else None)
    in_names, out_names, out_avals, zero_outs = [], [], [], []
    for alloc in nc.m.functions[0].allocations:
        if not isinstance(alloc, mybir.MemoryLocationSet):
            continue
        name = alloc.memorylocations[0].name
        if alloc.kind == "ExternalInput":
            if name != partition_name:
                in_names.append(name)
        elif alloc.kind == "ExternalOutput":
            out_names.append(name)
            shape = tuple(alloc.tensor_shape)
            dtype = mybir.dt.np(alloc.dtype)
            out_avals.append(jax.core.ShapedArray(shape, dtype))
            zero_outs.append(np.zeros(shape, dtype))
    n_params = len(in_names)
    n_outs = len(out_avals)
    all_in_names = list(in_names) + list(out_names)
    if partition_name is not None:
        all_in_names.append(partition_name)
    donate = tuple(range(n_params, n_params + n_outs))

    def _body(*args):
        operands = list(args)
        if partition_name is not None:
            operands.append(bass2jax.partition_id_tensor())
        outs = bass2jax._bass_exec_p.bind(
            *operands,
            out_avals=tuple(out_avals),
            in_names=tuple(all_in_names),
            out_names=tuple(out_names),
            lowering_input_output_aliases=(),
            sim_require_finite=True,
            sim_require_nnan=True,
            nc=nc,
        )
        return tuple(outs)

    devices = jax.devices()[:N_CORES]
    mesh = Mesh(np.asarray(devices), ("core",))
    in_specs = (PartitionSpec("core"),) * (n_params + n_outs)
    out_specs = (PartitionSpec("core"),) * n_outs
    sharded = jax.jit(
        shard_map(_body, mesh=mesh, in_specs=in_specs, out_specs=out_specs,
                  check_rep=False),
        donate_argnums=donate, keep_unused=True)

    def runner(in_maps):
        concat_in = [
            np.concatenate([np.asarray(m[name]) for m in in_maps], axis=0)
            for name in in_names
        ]
        concat_zeros = [
            np.zeros((N_CORES * z.shape[0], *z.shape[1:]), z.dtype)
            for z in zero_outs
        ]
        out_arrs = sharded(*concat_in, *concat_zeros)
        return [
            {name: np.asarray(out_arrs[i]).reshape(N_CORES, *out_avals[i].shape)[c]
             for i, name in enumerate(out_names)}
            for c in range(N_CORES)
        ]

    _CACHE["runner"] = runner
    _CACHE["nc"] = nc
    return runner


def make_in_maps(x, qkv_w, proj_w, proj_b):
    bf16 = ml_dtypes.bfloat16
    qkv_w = np.ascontiguousarray(np.asarray(qkv_w, dtype=np.float32))
    # reorder Q|K columns into per-head-pair [Q_hp(128) | K_hp(128)] blocks
    wqk_i = np.empty((C, 2 * C), dtype=np.float32)
    for hp in range(CT):
        wqk_i[:, hp * 256:hp * 256 + 128] = qkv_w[:, hp * 128:(hp + 1) * 128]
        wqk_i[:, hp * 256 + 128:(hp + 1) * 256] = \
            qkv_w[:, C + hp * 128:C + (hp + 1) * 128]
    wqk_i = wqk_i.astype(bf16)
    wv = qkv_w[:, 2 * C:3 * C].astype(bf16)
    pw_b = np.asarray(proj_w, dtype=np.float32).astype(bf16)
    pb = np.asarray(proj_b, dtype=np.float32).reshape(1, C)
    return [
        {
            "x_t": np.ascontiguousarray(
                np.asarray(x[b], dtype=np.float32).T).astype(bf16),
            "wqk": wqk_i,
            "wv": wv,
            "pw": pw_b,
            "pb": pb,
        }
        for b in range(N_CORES)
    ]


def kernel(x, qkv_w, proj_w, proj_b):
    runner = _get_runner()
    results = runner(make_in_maps(x, qkv_w, proj_w, proj_b))
    return np.stack([results[b]["out"] for b in range(N_CORES)], axis=0)


# revision 28
# speedup vs baseline: 1.5633x; 1.0358x over previous
"""Multi-head attention block (QKV proj + softmax attention + out proj) on 8
Trainium2 NeuronCores, data-parallel over the batch dimension (one batch
element per core).

Self-contained: hardcodes shapes for x [8, 1024, 768], qkv_w [768, 2304],
proj_w [768, 768], proj_b [768]; returns [8, 1024, 768] float32.

v2 design notes (why this is structured the way it is):
- bf16 matmul operands everywhere: weight loads use the background weight
  buffer / FWL so LDWEIGHTS hides under the matmul stream (f32r serializes
  ~180ns/MM), and input DMA bytes halve.
- The exp() of the 12.6M attention scores runs only on ScalarE
  ((N+352)/1.2ns per activate => ~110us total). That is co-bottleneck with
  the PE (~123us), so the QKV matmuls for head-pair hp+1 are emitted
  interleaved with the scores/AV of hp: ScalarE starts exping ~30us into
  the kernel and never starves.
- Softmax denominator comes for free as a 65th V_AUG row; the normalize is
  DVE reciprocal + GpSimd partition_broadcast + GpSimd multiply (the old
  DRAM-roundtrip broadcast exposed ~15us after the last head-pair).
- Scores matmuls use K=64 row tiles (head A rows 0-63, head B 64-127)
  emitted adjacently so both heads stream through the PE concurrently.
"""

import numpy as np
import ml_dtypes

import concourse.bass as bass
import concourse.mybir as mybir
import concourse.tile as tile
from concourse import bacc

N_CORES = 8
N = 1024          # tokens per batch element
C = 768           # model dim
H = 12            # heads
HD = 64           # head dim
CT = C // 128     # 6 contraction tiles
TT = N // 128     # 8 token tiles
SCALE = HD ** -0.5

F32 = mybir.dt.float32
BF16 = mybir.dt.bfloat16


def _build():
    nc = bacc.Bacc("TRN2", target_bir_lowering=False, debug=False,
                   num_devices=N_CORES)
    x_t = nc.dram_tensor("x_t", [C, N], BF16, kind="ExternalInput").ap()
    # wqk columns host-reordered: per head-pair hp, cols [hp*256, hp*256+128)
    # are Q features, [hp*256+128, (hp+1)*256) are K features.
    wqk = nc.dram_tensor("wqk", [C, 2 * C], BF16, kind="ExternalInput").ap()
    wv = nc.dram_tensor("wv", [C, C], BF16, kind="ExternalInput").ap()
    pw = nc.dram_tensor("pw", [C, C], BF16, kind="ExternalInput").ap()
    pb = nc.dram_tensor("pb", [1, C], mybir.dt.float32r, kind="ExternalInput").ap()
    out = nc.dram_tensor("out", [N, C], F32, kind="ExternalOutput").ap()

    with tile.TileContext(nc) as tc:
        _emit(nc, tc, x_t, wqk, wv, pw, pb, out)
    nc.compile()
    return nc


def _emit(nc, tc, x_t, wqk, wv, pw, pb, out):
    from contextlib import ExitStack
    ctx = ExitStack()
    with ctx:
        xt_pool = ctx.enter_context(tc.tile_pool(name="xt", bufs=1))
        wqk_pool = ctx.enter_context(tc.tile_pool(name="wqk", bufs=1))
        wv_pool = ctx.enter_context(tc.tile_pool(name="wv", bufs=1))
        pw_pool = ctx.enter_context(tc.tile_pool(name="pw", bufs=1))
        qk_pool = ctx.enter_context(tc.tile_pool(name="qk", bufs=1))
        vaug_pool = ctx.enter_context(tc.tile_pool(name="vaug", bufs=1))
        outt_pool = ctx.enter_context(tc.tile_pool(name="outt", bufs=1))
        exps_pool = ctx.enter_context(tc.tile_pool(name="exps", bufs=14))
        nrm_pool = ctx.enter_context(tc.tile_pool(name="nrm", bufs=2))
        rcp_pool = ctx.enter_context(tc.tile_pool(name="rcp", bufs=2))
        bc_pool = ctx.enter_context(tc.tile_pool(name="bc", bufs=2))
        const_pool = ctx.enter_context(tc.tile_pool(name="const", bufs=1))
        outsb_pool = ctx.enter_context(tc.tile_pool(name="outsb", bufs=8))
        dram_pool = ctx.enter_context(tc.tile_pool(name="drs", bufs=4, space="DRAM"))

        # ---- input DMAs ----
        # The three DMA queues together saturate HBM (~360GB/s), so order by
        # need-time: XT h0 + WQK g0 gate the first QK matmuls (~13us), XT h1
        # the qc=1 groups, Wv the V matmuls (~20us); everything else is late.
        XT = xt_pool.tile([128, CT, N], BF16, tag="xt")
        WQK = wqk_pool.tile([128, CT, 2 * C], BF16, tag="wqk")
        WV = wv_pool.tile([128, CT, C], BF16, tag="wv")
        PW = pw_pool.tile([128, CT, C], BF16, tag="pw")

        def _xt_dma(eng, h):
            eng.dma_start(
                XT[:, :, h * 512:(h + 1) * 512],
                x_t[:, h * 512:(h + 1) * 512].rearrange("(c p) n -> p c n", p=128))

        def _wqk_dma(eng, g):
            eng.dma_start(
                WQK[:, :, g * 512:(g + 1) * 512],
                wqk[:, g * 512:(g + 1) * 512].rearrange("(c p) f -> p c f", p=128))

        _xt_dma(nc.scalar, 0)
        _xt_dma(nc.sync, 1)
        _wqk_dma(nc.gpsimd, 0)
        for ct in range(CT):
            nc.scalar.dma_start(WV[:, ct, :], wv[ct * 128:(ct + 1) * 128, :])
        _wqk_dma(nc.sync, 1)
        _wqk_dma(nc.gpsimd, 2)
        nc.gpsimd.dma_start(PW[:], pw.rearrange("(c p) f -> p c f", p=128))
        # bias row + a ones column, f32r, for the bias-init matmul in proj
        pb_sb = const_pool.tile([1, C], mybir.dt.float32r, tag="pbsb")
        nc.gpsimd.dma_start(pb_sb[:], pb[:, :])
        ones_f32 = const_pool.tile([1, 128], F32, tag="onesf32")
        nc.vector.memset(ones_f32[:], 1.0)
        ones_fr = const_pool.tile([1, 128], mybir.dt.float32r, tag="onesfr")
        nc.vector.tensor_copy(ones_fr[:], ones_f32[:])

        ones_bf = const_pool.tile([128, 96], BF16, tag="ones")
        nc.vector.memset(ones_bf[:], 1.0)
        V_AUG = vaug_pool.tile([128, TT, H, HD + 1], BF16, tag="vaug")
        nc.vector.tensor_copy(
            V_AUG[:, :, :, HD:HD + 1].rearrange("p t h one -> p (t h one)"),
            ones_bf[:])

        QT = qk_pool.tile([128, CT, N], BF16, tag="qt")
        KT = qk_pool.tile([128, CT, N], BF16, tag="kt")
        outT = outt_pool.tile([128, CT, N], BF16, tag="outt")

        sc_ps = ctx.enter_context(tc.tile_pool(name="scps", bufs=2, space="PSUM"))
        av_ps = ctx.enter_context(tc.tile_pool(name="avps", bufs=2, space="PSUM"))

        # PE warmup: ~16 dummy matmuls on a constant tile while the input
        # DMAs stream in. Sustained PE activity flips the HAM clock gate to
        # K=8/8 (~3.4us busy window) so the first real matmuls run at
        # 2.4GHz instead of 1.2GHz.
        warm = const_pool.tile([128, 512], BF16, tag="warm")
        nc.vector.memset(warm[:], 0.5)
        wps = av_ps.tile([128, 1024], F32, tag="av")
        for _ in range(36):
            nc.tensor.matmul(wps[:, 0:512], lhsT=warm[:, 0:128],
                             rhs=warm[:], start=True, stop=True)
        # preload the exp activation table set (~2.7us) during the DMA wait
        # so the first real exp doesn't pay it
        act_warm = const_pool.tile([1, 16], F32, tag="actwarm")
        nc.scalar.activation(act_warm[:], ones_f32[0:1, 0:16],
                             mybir.ActivationFunctionType.Exp, scale=1.0)

        # ---- emit helpers ----
        def emit_v(tt):
            # V matmuls use the av PSUM pool: keeps the scores pool rotation
            # free of V-eviction dependencies, and the av slots aren't needed
            # until the first AV accumulation (after all V tiles retire).
            ps = av_ps.tile([128, 1024], F32, tag="av")
            for w0, wn in ((0, 512), (512, 256)):
                for ct in range(CT):
                    nc.tensor.matmul(
                        ps[:, w0:w0 + wn],
                        lhsT=XT[:, ct, tt * 128:(tt + 1) * 128],
                        rhs=WV[:, ct, w0:w0 + wn],
                        start=(ct == 0), stop=(ct == CT - 1))
            nc.vector.tensor_copy(
                V_AUG[:, tt, :, 0:HD],
                ps[:, :C].rearrange("p (h d) -> p h d", d=HD))

        def emit_qk(hp, which):
            # which: 0 => Q chunk of head-pair hp, 1 => K chunk
            dest = QT if which == 0 else KT
            f0 = hp * 256 + which * 128
            ps = sc_ps.tile([128, 1024], F32, tag="sc")
            # qc-outer: the qc=0 group only needs the first XT token half,
            # so the very first matmuls start ~2.5us earlier at kernel start
            for qc in range(2):
                for ct in range(CT):
                    nc.tensor.matmul(
                        ps[:, qc * 512:(qc + 1) * 512],
                        lhsT=WQK[:, ct, f0:f0 + 128],
                        rhs=XT[:, ct, qc * 512:(qc + 1) * 512],
                        start=(ct == 0), stop=(ct == CT - 1))
            nc.vector.tensor_copy(dest[:, hp, :], ps[:])

        def emit_scores(hp, kt):
            # One PSUM tile per query-half holding BOTH heads' scores
            # (A in cols 0:512 from PE rows 0-63, B in 512:1024 from rows
            # 64-127). A single exp gates both heads' next-kt matmuls, so
            # the A/B pair issues adjacently and streams concurrently
            # through disjoint row groups of the PE array.
            es = []
            for qc in range(2):
                ps = sc_ps.tile([128, 1024], F32, tag="sc")
                nc.tensor.matmul(
                    ps[:, 0:512],
                    lhsT=KT[0:64, hp, kt * 128:(kt + 1) * 128],
                    rhs=QT[0:64, hp, qc * 512:(qc + 1) * 512],
                    start=True, stop=True)
                nc.tensor.matmul(
                    ps[:, 512:1024],
                    lhsT=KT[64:128, hp, kt * 128:(kt + 1) * 128],
                    rhs=QT[64:128, hp, qc * 512:(qc + 1) * 512],
                    start=True, stop=True)
                e = exps_pool.tile([128, N], BF16, tag="exps")
                nc.scalar.activation(e[:], ps[:], mybir.ActivationFunctionType.Exp,
                                     scale=SCALE)
                es.append(e)
            return es[0], es[1]

        def emit_av(hp, kt, e0, e1, avA, avB):
            for av, off, head in ((avA, 0, 2 * hp), (avB, 512, 2 * hp + 1)):
                for qc, e in ((0, e0), (1, e1)):
                    nc.tensor.matmul(
                        av[:, qc * 512:(qc + 1) * 512],
                        lhsT=V_AUG[:, kt, head, :],
                        rhs=e[:, off:off + 512],
                        start=(kt == 0), stop=(kt == TT - 1))

        def emit_normalize_pre(hp, avA, avB):
            # Denominator reciprocal via the DRAM-roundtrip reshape
            # ([1,1024] -> [64,16] so the DVE reciprocal runs 64 lanes wide)
            # plus a stride-0 DRAM->SBUF broadcast: all ops in this chain are
            # HW-proven. The chain (~4.5us latency) rides the idle gpsimd
            # queue and only the last head-pair exposes it.
            Us, Rts, bcs = [], [], []
            for av in (avA, avB):
                U = nrm_pool.tile([HD + 1, N], F32, tag="U")
                nc.vector.tensor_copy(U[:], av[:])
                dscr = dram_pool.tile([N], F32, tag="dscr")
                nc.sync.dma_start(dscr[:], U[HD:HD + 1, :])
                Dt = rcp_pool.tile([64, 16], F32, tag="Dt")
                nc.sync.dma_start(Dt[:], dscr[:].rearrange("(p j) -> p j", j=16))
                Us.append((U, Dt))
            for U, Dt in Us:
                Rt = rcp_pool.tile([64, 16], F32, tag="Rt")
                scr = rcp_pool.tile([64, 16], F32, tag="scr")
                nc.vector.reciprocal_approx_accurate(Rt[:], Dt[:], scr[:])
                Rts.append(Rt)
            for Rt in Rts:
                rscr = dram_pool.tile([N], F32, tag="rscr")
                nc.sync.dma_start(rscr[:].rearrange("(p j) -> p j", j=16), Rt[:])
                bcst = bc_pool.tile([64, N], F32, tag="bc")
                rs = rscr[:]
                bcast_ap = bass.AP(tensor=rs.tensor, offset=rs.offset,
                                   ap=[[0, 64]] + [list(a) for a in rs.ap])
                nc.sync.dma_start(bcst[:], bcast_ap)
                bcs.append(bcst)
            return Us, bcs

        def emit_normalize_muls(hp, Us, bcs):
            for (U, _), bcst, poff in ((Us[0], bcs[0], 0), (Us[1], bcs[1], 64)):
                nc.vector.tensor_mul(
                    outT[poff:poff + 64, hp, :], U[0:HD, :], bcst[:])

        def emit_normalize(hp, avA, avB):
            Us, bcs = emit_normalize_pre(hp, avA, avB)
            emit_normalize_muls(hp, Us, bcs)

        # proj is split: partials (bias + head-pairs 0-4) run while the last
        # head-pair's normalize chain is in flight, with ScalarE (idle after
        # the final exp) doing the PSUM->SBUF evictions; only the ct=5
        # contribution remains after the last normalize multiply.
        def emit_proj_partial(tt):
            ps = sc_ps.tile([128, 1024], F32, tag="sc")
            for nch in range(2):
                nc.tensor.matmul(
                    ps[:, nch * 512:nch * 512 + 384],
                    lhsT=ones_fr[0:1, :],
                    rhs=pb_sb[0:1, nch * 384:(nch + 1) * 384],
                    start=True, stop=False)
            for ct in range(CT - 1):
                for nch in range(2):
                    nc.tensor.matmul(
                        ps[:, nch * 512:nch * 512 + 384],
                        lhsT=outT[:, ct, tt * 128:(tt + 1) * 128],
                        rhs=PW[:, ct, nch * 384:(nch + 1) * 384],
                        start=False, stop=(ct == CT - 2))
            osb = outsb_pool.tile([128, C], F32, tag="outsb")
            ps_v = ps[:].rearrange("p (c x) -> p c x", c=2)[:, :, 0:384]
            osb_v = osb[:].rearrange("p (c x) -> p c x", c=2)
            nc.scalar.activation(osb_v, ps_v, mybir.ActivationFunctionType.Copy)
            return osb

        def emit_proj_final(tt, osb):
            ps = sc_ps.tile([128, 1024], F32, tag="sc")
            for nch in range(2):
                nc.tensor.matmul(
                    ps[:, nch * 512:nch * 512 + 384],
                    lhsT=outT[:, CT - 1, tt * 128:(tt + 1) * 128],
                    rhs=PW[:, CT - 1, nch * 384:(nch + 1) * 384],
                    start=True, stop=True)
            ps_v = ps[:].rearrange("p (c x) -> p c x", c=2)[:, :, 0:384]
            osb_v = osb[:].rearrange("p (c x) -> p c x", c=2)
            nc.vector.tensor_add(osb_v, osb_v, ps_v)
            eng = nc.sync if tt % 2 == 0 else nc.gpsimd
            eng.dma_start(out[tt * 128:(tt + 1) * 128, :], osb[:])

        # ---- emission schedule ----
        # QK for head-pair 0 first so ScalarE's exp stream starts ~18us in;
        # the V matmuls interleave into hp0's score iterations (kts 0-3).
        # AV for hp0 is held back (pend>=5) until all V tiles are emitted,
        # else the AV matmul would head-block the PE FIFO on V_AUG writes
        # queued behind it.
        emit_qk(0, 0)
        emit_qk(0, 1)
        for hp in range(CT):
            avA = avB = None
            pend = []
            thresh = 5 if hp == 0 else 2
            for kt in range(TT):
                eA, eB = emit_scores(hp, kt)
                pend.append((hp, kt, eA, eB))
                if hp == 0:
                    if kt < 4:
                        emit_v(2 * kt)
                        emit_v(2 * kt + 1)
                    if kt == 4 and hp + 1 < CT:
                        emit_qk(hp + 1, 0)
                    if kt == 6 and hp + 1 < CT:
                        emit_qk(hp + 1, 1)
                else:
                    if kt == 1 and hp + 1 < CT:
                        emit_qk(hp + 1, 0)
                    if kt == 6 and hp + 1 < CT:
                        # at kt6 so the QK matmuls give the PE work across the
                        # head-pair boundary while the last exps drain
                        emit_qk(hp + 1, 1)
                if len(pend) >= thresh:
                    if avA is None:
                        avA = av_ps.tile([HD + 1, 1024], F32, tag="av")
                        avB = av_ps.tile([HD + 1, 1024], F32, tag="av")
                    emit_av(*pend.pop(0), avA, avB)
            if avA is None:
                avA = av_ps.tile([HD + 1, 1024], F32, tag="av")
                avB = av_ps.tile([HD + 1, 1024], F32, tag="av")
            for args in pend:
                emit_av(*args, avA, avB)
            if hp < CT - 1:
                emit_normalize(hp, avA, avB)
            else:
                Us, bcs = emit_normalize_pre(hp, avA, avB)
                osbs = [emit_proj_partial(tt) for tt in range(TT)]
                emit_normalize_muls(hp, Us, bcs)
                for tt in range(TT):
                    emit_proj_final(tt, osbs[tt])


_CACHE = {}


def _get_runner():
    """Build + compile once; return a callable(in_maps) -> list of out dicts.

    Keeps a persistent jitted shard_map executable so repeat calls skip
    retracing/recompiling (mirrors bass2jax.run_bass_via_pjrt).
    """
    if "runner" in _CACHE:
        return _CACHE["runner"]

    import jax
    from jax.experimental.shard_map import shard_map
    from jax.sharding import Mesh, PartitionSpec
    from concourse import bass2jax

    nc = _build()
    bass2jax.install_neuronx_cc_hook()

    partition_name = (nc.partition_id_tensor.name if nc.partition_id_tensor
                      else None)
    in_names, out_names, out_avals, zero_outs = [], [], [], []
    for alloc in nc.m.functions[0].allocations:
        if not isinstance(alloc, mybir.MemoryLocationSet):
            continue
        name = alloc.memorylocations[0].name
        if alloc.kind == "ExternalInput":
            if name != partition_name:
                in_names.append(name)
        elif alloc.kind == "ExternalOutput":
            out_names.append(name)
            shape = tuple(alloc.tensor_shape)
            dtype = mybir.dt.np(alloc.dtype)
            out_avals.append(jax.core.ShapedArray(shape, dtype))
            zero_outs.append(np.zeros(shape, dtype))
    n_params = len(in_names)
    n_outs = len(out_avals)
    all_in_names = list(in_names) + list(out_names)
    if partition_name is not None:
        all_in_names.append(partition_name)
    donate = tuple(range(n_params, n_params + n_outs))

    def _body(*args):
        operands = list(args)
        if partition_name is not None:
            operands.append(bass2jax.partition_id_tensor())
        outs = bass2jax._bass_exec_p.bind(
            *operands,
            out_avals=tuple(out_avals),
            in_names=tuple(all_in_names),
            out_names=tuple(out_names),
            lowering_input_output_aliases=(),
            sim_require_finite=True,
            sim_require_nnan=True,
            nc=nc,
        )
        return tuple(outs)

    devices = jax.devices()[:N_CORES]
    mesh = Mesh(np.asarray(devices), ("core",))
    in_specs = (PartitionSpec("core"),) * (n_params + n_outs)
    out_specs = (PartitionSpec("core"),) * n_outs
    sharded = jax.jit(
        shard_map(_body, mesh=mesh, in_specs=in_specs, out_specs=out_specs,
                  check_rep=False),
        donate_argnums=donate, keep_unused=True)

    def runner(in_maps):
        concat_in = [
            np.concatenate([np.asarray(m[name]) for m in in_maps], axis=0)
            for name in in_names
        ]
        concat_zeros = [
            np.zeros((N_CORES * z.shape[0], *z.shape[1:]), z.dtype)
            for z in zero_outs
        ]
        out_arrs = sharded(*concat_in, *concat_zeros)
        return [
            {name: np.asarray(out_arrs[i]).reshape(N_CORES, *out_avals[i].shape)[c]
             for i, name in enumerate(out_names)}
            for c in range(N_CORES)
        ]

    _CACHE["runner"] = runner
    _CACHE["nc"] = nc
    return runner


def make_in_maps(x, qkv_w, proj_w, proj_b):
    bf16 = ml_dtypes.bfloat16
    qkv_w = np.ascontiguousarray(np.asarray(qkv_w, dtype=np.float32))
    # reorder Q|K columns into per-head-pair [Q_hp(128) | K_hp(128)] blocks
    wqk_i = np.empty((C, 2 * C), dtype=np.float32)
    for hp in range(CT):
        wqk_i[:, hp * 256:hp * 256 + 128] = qkv_w[:, hp * 128:(hp + 1) * 128]
        wqk_i[:, hp * 256 + 128:(hp + 1) * 256] = \
            qkv_w[:, C + hp * 128:C + (hp + 1) * 128]
    wqk_i = wqk_i.astype(bf16)
    wv = qkv_w[:, 2 * C:3 * C].astype(bf16)
    pw_b = np.asarray(proj_w, dtype=np.float32).astype(bf16)
    pb = np.asarray(proj_b, dtype=np.float32).reshape(1, C)
    return [
        {
            "x_t": np.ascontiguousarray(
                np.asarray(x[b], dtype=np.float32).T).astype(bf16),
            "wqk": wqk_i,
            "wv": wv,
            "pw": pw_b,
            "pb": pb,
        }
        for b in range(N_CORES)
    ]


def kernel(x, qkv_w, proj_w, proj_b):
    runner = _get_runner()
    results = runner(make_in_maps(x, qkv_w, proj_w, proj_b))
    return np.stack([results[b]["out"] for b in range(N_CORES)], axis=0)
